# revision 3
# baseline (speedup 1.0000x reference)
"""BitNet FFN Trainium2 kernel — 8-core tensor-parallel over d_ff.

Math (forward values of the STE reference):
  wq(w)  = clip(round(w/s), -1, 1) * s,  s = mean(|w|) + EPS        (ternary)
  xq(x)  = round(x/sx) * sx,  sx = max(absmax_row(x), EPS)/127      (int8 range)
  gate = sigmoid(xq @ wq_g.T); up = xq @ wq_u.T; h = gate*up
  out  = hq(h) @ wq_d.T

Sharding: w_gate/w_up rows and w_down columns are sharded 8 ways (ff_local
= d_ff/8 per core), so each core's ternarized weights live in SBUF for the
whole kernel (no weight streaming).  Tokens are processed in 8 chunks of
T/8; every core computes gate/up/h' for ALL tokens against its ff shard.
Each core quantizes its own token shard of x and AllGathers the transposed
bf16 result piece-by-piece (piece-contiguous DRAM layout keeps every DMA
packet large), so chunk i's xqT is exactly AllGather block i.

Per chunk: G/U int matmuls -> sigmoid*U -> local per-token absmax ->
tiny AllReduce(max) for the exact global h scale -> quantize h' ->
down-proj partial matmul (scaled by s_h*s_wd per token, fused into the
PSUM drain) -> f32 ReduceScatter over tokens.  All matmuls are bf16 on
exact integers (|int|<=127 activations, ternary weights) with fp32 PSUM
accumulation, so integer matmuls are exact; scales fold in fp32 outside.

All transposes run on the tensor engine (identity matmul) — DMA
transposes emit 256B packets that flood the DMA queues.  Collectives
serialize on the gpsimd queue and block it while running, so the
schedule keeps that queue empty right before each chunk's absmax-AR and
gives the AR result three tiles of slack before anything consumes it.
"""

import sys

sys.path.insert(0, "/opt/trn_rl_repo")

import contextlib

import numpy as np

import concourse.tile as tile
from concourse import bacc, mybir
from concourse.masks import make_identity

F32 = mybir.dt.float32
BF16 = mybir.dt.bfloat16
ADD = mybir.AluOpType.add
SUB = mybir.AluOpType.subtract
MULT = mybir.AluOpType.mult
MAX = mybir.AluOpType.max
BYP = mybir.AluOpType.bypass
AXX = mybir.AxisListType.X
AFT = mybir.ActivationFunctionType

EPS = 1e-5
CR = 12582912.0  # 1.5*2^23: fp32 RNE round-to-integer magic constant
ALPHA = 1.0986122886681098  # atanh(0.5)/0.5 : tanh(ALPHA*0.5) == 0.5
P = 128
W = 512  # matmul moving free dim (one PSUM bank of f32)


def build_program(TT, DM, FF, NC):
    """TT: total tokens; DM: d_model; FF: d_ff; NC: cores."""
    TC = TT // NC            # tokens per chunk == tokens per x-shard
    MT = TC // P             # token tiles per chunk
    KD = DM // P             # d_model k-blocks
    FFL = FF // NC           # local ff shard
    FFK = FFL // P           # ff k-blocks (phase 3 contraction)
    NFH = max(1, FFL // W)   # ff halves per G/U psum group
    WF = min(W, FFL)
    ND3 = max(1, DM // W)    # dm quarters of the P3 output
    W3 = min(W, DM)
    NPC = min(4, MT)         # xqT AllGather pieces per chunk
    TPQ = TC // NPC
    MTQ = TPQ // P           # token tiles per piece
    NW = float(FF * DM)      # elements per full weight matrix
    TC8 = TC // NC           # RS output rows per chunk per core
    rg = [list(range(NC))]
    assert FFK <= 8 and TPQ % P == 0

    nc = bacc.Bacc(
        "TRN2",
        target_bir_lowering=False,
        debug=False,
        enable_asserts=False,
        num_devices=NC,
    )

    x_d = nc.dram_tensor("x", [TC, DM], F32, kind="ExternalInput")
    wg_d = nc.dram_tensor("wg", [FFL, DM], F32, kind="ExternalInput")
    wu_d = nc.dram_tensor("wu", [FFL, DM], F32, kind="ExternalInput")
    wd_d = nc.dram_tensor("wd", [DM, FFL], F32, kind="ExternalInput")
    out_d = nc.dram_tensor("out_t", [NC * TC8, DM], F32, kind="ExternalOutput")

    with tile.TileContext(nc, num_cores=NC) as tc:
        with contextlib.ExitStack() as outer:
            dram = outer.enter_context(tc.tile_pool(name="dram", bufs=1, space="DRAM"))
            tiny = outer.enter_context(tc.tile_pool(name="tiny", bufs=1))

            # DRAM scratch (xqT is piece-major so every DMA packet is big)
            xqt_sh = [
                dram.tile([P, KD, TPQ], BF16, name=f"xqt_sh{q}") for q in range(NPC)
            ]
            xqt_all = [
                dram.tile([NC, P, KD, TPQ], BF16, name=f"xqt_all{q}",
                          addr_space="Shared")
                for q in range(NPC)
            ]
            sx_sh_d = dram.tile([P, MT], F32)
            sx_all_d = dram.tile([NC, P, MT], F32, addr_space="Shared")
            ws_in = dram.tile([1, 4], F32)
            ws_out = dram.tile([1, 4], F32, addr_space="Shared")
            hmax_in = dram.tile([NC, P, MT], F32)
            hmax_out = [
                dram.tile([P, MT], F32, name=f"hmax_out{i}", addr_space="Shared")
                for i in range(NC)
            ]
            pout_d = dram.tile([NC, TC, DM], F32)
            rsout_d = dram.tile([NC, TC8, DM], F32)

            # persistent small tiles
            ones_row = tiny.tile([1, P], F32)
            nc.vector.memset(ones_row, 1.0)
            ident = tiny.tile([P, P], BF16)
            make_identity(nc, ident)
            sb = tiny.tile([P, 8], F32)   # bcast: bgA,buA,bdA,-,swg,swu,swd,-
            NTT = NC * MT                 # total token tiles
            sx_sb = tiny.tile([P, NTT], F32)
            sxg_sb = tiny.tile([P, NTT], F32)
            sxu_sb = tiny.tile([P, NTT], F32)
            rh_sb = tiny.tile([P, NTT], F32)
            shd_sb = tiny.tile([P, NTT], F32)
            hmax_sb = tiny.tile([P, NTT], F32)

            # persistent ternary weights (bf16, transposed for matmul)
            wgt_sb = tiny.tile([P, KD, FFL], BF16)
            wut_sb = tiny.tile([P, KD, FFL], BF16)
            wdt_sb = tiny.tile([P, FFK, DM], BF16)

            def pe_transpose(src, nblk, dst3, ps_pool):
                """src [P, nblk*P] bf16 -> dst3 [P, nblk, P] (3D slice),
                via PE-transpose through PSUM in groups of <=8 blocks."""
                for h0 in range(0, nblk, 8):
                    nb = min(8, nblk - h0)
                    ps = ps_pool.tile([P, 8, P], BF16, name="ps_tr")
                    for j in range(nb):
                        nc.tensor.transpose(
                            ps[:, j, :], src[:, (h0 + j) * P : (h0 + j + 1) * P],
                            ident,
                        )
                    nc.vector.tensor_copy(
                        dst3[:, h0 : h0 + nb, :], ps[:, :nb, :]
                    )

            # ------------- prologue: x-quant shard + AG, weight scales,
            # ------------- ternarize weights into SBUF
            with contextlib.ExitStack() as pro:
                pspro = pro.enter_context(
                    tc.tile_pool(name="pspro", bufs=3, space="PSUM")
                )
                ps0 = pro.enter_context(
                    tc.tile_pool(name="ps0", bufs=1, space="PSUM")
                )
                xw_p = pro.enter_context(tc.tile_pool(name="xw", bufs=2))
                xtr_p = pro.enter_context(tc.tile_pool(name="xtr", bufs=2))
                s0_p = pro.enter_context(tc.tile_pool(name="s0", bufs=3))
                s0t_p = pro.enter_context(tc.tile_pool(name="s0t", bufs=4))
                wr_p = pro.enter_context(tc.tile_pool(name="wr", bufs=3))
                wt_p = pro.enter_context(tc.tile_pool(name="wt", bufs=2))

                # S0: weight scale sums (|w| over shard, AllReduce later)
                acc3 = tiny.tile([P, 4], F32)
                nc.vector.memset(acc3, 0.0)
                for src, col, rows, cols in (
                    (wg_d, 0, FFL, DM),
                    (wu_d, 1, FFL, DM),
                    (wd_d, 2, DM, FFL),
                ):
                    for r0 in range(0, rows, P):
                        t_in = s0_p.tile([P, DM], F32, name="s0raw")
                        nc.sync.dma_start(t_in[:, :cols], src[r0 : r0 + P, :])
                        t_sum = s0t_p.tile([P, 1], F32, name="s0sum")
                        nc.scalar.activation(
                            out=t_in[:, :cols], in_=t_in[:, :cols],
                            func=AFT.Abs, accum_out=t_sum,
                        )
                        nc.vector.tensor_tensor(
                            out=acc3[:, col : col + 1],
                            in0=acc3[:, col : col + 1],
                            in1=t_sum,
                            op=ADD,
                        )
                ones_col = s0t_p.tile([P, 1], F32, name="ones_col")
                nc.vector.memset(ones_col, 1.0)
                ps_s = ps0.tile([P, W], F32, name="ps_s")
                nc.tensor.matmul(
                    ps_s[:4, :1], acc3[:, :4], ones_col, start=True, stop=True
                )
                sb_s = s0t_p.tile([4, 1], F32, name="sb_s")
                nc.vector.tensor_copy(sb_s, ps_s[:4, :1])
                nc.gpsimd.dma_start(ws_in[0, :4], sb_s[:, 0])
                nc.gpsimd.collective_compute(
                    "AllReduce",
                    ADD,
                    replica_groups=rg,
                    ins=[ws_in[:].opt()],
                    outs=[ws_out[:].opt()],
                )
                sums_row = s0t_p.tile([1, 4], F32, name="sums_row")
                nc.gpsimd.dma_start(sums_row, ws_out[:])
                sw_row = s0t_p.tile([1, 4], F32, name="sw_row")
                nc.vector.tensor_scalar(
                    out=sw_row, in0=sums_row, scalar1=1.0 / NW, scalar2=EPS,
                    op0=MULT, op1=ADD,
                )
                beta_row = s0t_p.tile([1, 4], F32, name="beta_row")
                nc.vector.reciprocal(beta_row, sw_row)
                row8 = s0t_p.tile([1, 8], F32, name="row8")
                nc.vector.tensor_scalar(
                    out=row8[:, 0:4], in0=beta_row, scalar1=ALPHA, scalar2=None,
                    op0=MULT, op1=BYP,
                )
                nc.vector.tensor_copy(row8[:, 4:8], sw_row)
                ps_b = ps0.tile([P, W], F32, name="ps_b")
                nc.tensor.matmul(ps_b[:, :8], ones_row, row8, start=True, stop=True)
                nc.vector.tensor_copy(sb, ps_b[:, :8])

                # X-quant of own token shard -> transposed bf16 pieces -> AG
                sxl = tiny.tile([P, MT], F32)
                xtr_tiles = {}
                for m in range(MT):
                    q, mrel = m // MTQ, m % MTQ
                    if mrel == 0:
                        xtr_tiles[q] = xtr_p.tile([P, KD, TPQ], BF16, name="xtr")
                    xt = xw_p.tile([P, DM], F32, name="xt")
                    nc.sync.dma_start(xt, x_d[m * P : (m + 1) * P, :])
                    amax = s0t_p.tile([P, 1], F32, name="amax")
                    nc.vector.tensor_reduce(
                        amax, xt, axis=AXX, op=MAX, apply_absolute_value=True
                    )
                    nc.vector.tensor_scalar(
                        out=sxl[:, m : m + 1], in0=amax, scalar1=EPS,
                        scalar2=1.0 / 127.0, op0=MAX, op1=MULT,
                    )
                    rxc = s0t_p.tile([P, 1], F32, name="rxc")
                    nc.vector.reciprocal(rxc, sxl[:, m : m + 1])
                    nc.vector.tensor_scalar(
                        out=xt, in0=xt, scalar1=rxc, scalar2=CR, op0=MULT, op1=ADD,
                    )
                    xq = xw_p.tile([P, DM], BF16, name="xq")
                    nc.vector.tensor_scalar(
                        out=xq, in0=xt, scalar1=CR, scalar2=None, op0=SUB, op1=BYP,
                    )
                    for h0 in range(0, KD, 8):
                        nb = min(8, KD - h0)
                        ps = pspro.tile([P, 8, P], BF16, name="ps_tr")
                        for j in range(nb):
                            nc.tensor.transpose(
                                ps[:, j, :], xq[:, (h0 + j) * P : (h0 + j + 1) * P],
                                ident,
                            )
                        nc.vector.tensor_copy(
                            xtr_tiles[q][:, h0 : h0 + nb, mrel * P : (mrel + 1) * P],
                            ps[:, :nb, :],
                        )
                    if mrel == MTQ - 1:
                        nc.sync.dma_start(xqt_sh[q][:], xtr_tiles[q])
                nc.gpsimd.dma_start(sx_sh_d[:], sxl)
                nc.gpsimd.collective_compute(
                    "AllGather",
                    BYP,
                    replica_groups=rg,
                    ins=[sx_sh_d[:].opt()],
                    outs=[sx_all_d[:].opt()],
                )
                # sx readback for all tokens -> per-partition columns
                # (before the big xqt AGs so sigmoid scales aren't gated
                # behind them on the serialized collective queue)
                for i in range(NC):
                    nc.sync.dma_start(
                        sx_sb[:, i * MT : (i + 1) * MT], sx_all_d[i]
                    )
                nc.vector.tensor_scalar(
                    out=sxg_sb, in0=sx_sb, scalar1=sb[:, 4:5], scalar2=None,
                    op0=MULT, op1=BYP,
                )
                nc.vector.tensor_scalar(
                    out=sxu_sb, in0=sx_sb, scalar1=sb[:, 5:6], scalar2=None,
                    op0=MULT, op1=BYP,
                )
                for q in range(NPC):
                    nc.gpsimd.collective_compute(
                        "AllGather",
                        BYP,
                        replica_groups=rg,
                        ins=[xqt_sh[q][:].opt()],
                        outs=[xqt_all[q][:].opt()],
                    )

                # ternarize weights into SBUF (transposed bf16, PE transpose)
                def ternarize(src, beta_col, dst, nblk, rows):
                    cols = nblk * P
                    for r0 in range(0, rows, P):
                        raw = wr_p.tile([P, DM], F32, name="wraw")
                        nc.sync.dma_start(raw[:, :cols], src[r0 : r0 + P, :])
                        nc.scalar.activation(
                            out=raw[:, :cols], in_=raw[:, :cols], func=AFT.Tanh,
                            scale=sb[:, beta_col : beta_col + 1],
                        )
                        tern = wt_p.tile([P, DM], BF16, name="wtern")
                        nc.vector.tensor_scalar(
                            out=tern[:, :cols], in0=raw[:, :cols], scalar1=CR,
                            scalar2=CR, op0=ADD, op1=SUB,
                        )
                        pe_transpose(tern, nblk, dst[:, :, r0 : r0 + P], pspro)

                ternarize(wg_d, 0, wgt_sb, KD, FFL)
                ternarize(wu_d, 1, wut_sb, KD, FFL)
                ternarize(wd_d, 2, wdt_sb, FFK, DM)

            # ------------- main pipeline over 8 token chunks -------------
            with contextlib.ExitStack() as mn:
                psum = mn.enter_context(
                    tc.tile_pool(name="psum", bufs=7, space="PSUM")
                )
                pstr = mn.enter_context(
                    tc.tile_pool(name="pstr", bufs=1, space="PSUM")
                )
                xqc_p = mn.enter_context(tc.tile_pool(name="xqc", bufs=2))
                hp_p = mn.enter_context(tc.tile_pool(name="hp", bufs=MT + 5))
                gt_p = mn.enter_context(tc.tile_pool(name="gt", bufs=2))
                hq_p = mn.enter_context(tc.tile_pool(name="hq", bufs=2))
                hqt_p = mn.enter_context(tc.tile_pool(name="hqt", bufs=4))
                st_p = mn.enter_context(tc.tile_pool(name="st", bufs=2))
                sc_p = mn.enter_context(tc.tile_pool(name="scp", bufs=4))

                hp_tiles = {}
                xqc_tiles = {}

                def load_xqc(i, q):
                    t = xqc_p.tile([P, KD, TPQ], BF16, name="xqc")
                    nc.sync.dma_start(t, xqt_all[q][i])
                    xqc_tiles[(i, q)] = t

                def phase1_tile(i, m):
                    g = i * MT + m
                    xq_t = xqc_tiles[(i, m // MTQ)]
                    trel = (m % MTQ) * P
                    psG = [psum.tile([P, W], F32, name="ps_main") for _ in range(NFH)]
                    psU = [psum.tile([P, W], F32, name="ps_main") for _ in range(NFH)]
                    for k in range(KD):
                        lhsT = xq_t[:, k, trel : trel + P]
                        st, sp = (k == 0), (k == KD - 1)
                        for f in range(NFH):
                            nc.tensor.matmul(
                                psG[f][:, :WF], lhsT,
                                wgt_sb[:, k, f * WF : (f + 1) * WF],
                                start=st, stop=sp,
                            )
                        for f in range(NFH):
                            nc.tensor.matmul(
                                psU[f][:, :WF], lhsT,
                                wut_sb[:, k, f * WF : (f + 1) * WF],
                                start=st, stop=sp,
                            )
                    hp = hp_p.tile([P, FFL], F32, name="hp")
                    for f in range(NFH):
                        gt = gt_p.tile([P, WF], F32, name="gt")
                        nc.scalar.activation(
                            out=gt, in_=psG[f][:, :WF], func=AFT.Sigmoid,
                            scale=sxg_sb[:, g : g + 1],
                        )
                        nc.vector.tensor_tensor(
                            out=hp[:, f * WF : (f + 1) * WF], in0=gt,
                            in1=psU[f][:, :WF], op=MULT,
                        )
                    nc.vector.tensor_reduce(
                        hmax_sb[:, g : g + 1], hp, axis=AXX, op=MAX,
                        apply_absolute_value=True,
                    )
                    hp_tiles[(i, m)] = hp

                def chunk_absmax_ar(i):
                    nc.gpsimd.dma_start(
                        hmax_in[i], hmax_sb[:, i * MT : (i + 1) * MT]
                    )
                    nc.gpsimd.collective_compute(
                        "AllReduce",
                        MAX,
                        replica_groups=rg,
                        ins=[hmax_in[i].opt()],
                        outs=[hmax_out[i][:].opt()],
                    )

                def chunk_scales(i):
                    # amg readback rides the sync queue so the scale chain
                    # never waits behind a ReduceScatter on gpsimd
                    amg = sc_p.tile([P, MT], F32, name="amg")
                    nc.sync.dma_start(amg, hmax_out[i][:])
                    cs = slice(i * MT, (i + 1) * MT)
                    ah = sc_p.tile([P, MT], F32, name="ah")
                    nc.vector.tensor_tensor(
                        out=ah, in0=amg, in1=sxu_sb[:, cs], op=MULT
                    )
                    sh = sc_p.tile([P, MT], F32, name="sh")
                    nc.vector.tensor_scalar(
                        out=sh, in0=ah, scalar1=EPS, scalar2=1.0 / 127.0,
                        op0=MAX, op1=MULT,
                    )
                    rs_t = sc_p.tile([P, MT], F32, name="rs_t")
                    nc.vector.reciprocal(rs_t, sh)
                    nc.vector.tensor_tensor(
                        out=rh_sb[:, cs], in0=rs_t, in1=sxu_sb[:, cs], op=MULT
                    )
                    nc.vector.tensor_scalar(
                        out=shd_sb[:, cs], in0=sh, scalar1=sb[:, 6:7], scalar2=None,
                        op0=MULT, op1=BYP,
                    )

                def quant_tile(i, m):
                    g = i * MT + m
                    hp = hp_tiles.pop((i, m))
                    nc.vector.tensor_scalar(
                        out=hp, in0=hp, scalar1=rh_sb[:, g : g + 1], scalar2=CR,
                        op0=MULT, op1=ADD,
                    )
                    hq = hq_p.tile([P, FFL], BF16, name="hq")
                    nc.vector.tensor_scalar(
                        out=hq, in0=hp, scalar1=CR, scalar2=None, op0=SUB, op1=BYP,
                    )
                    ps = pstr.tile([P, FFK, P], BF16, name="ps_hqt")
                    for j in range(FFK):
                        nc.tensor.transpose(
                            ps[:, j, :], hq[:, j * P : (j + 1) * P], ident
                        )
                    hqt = hqt_p.tile([P, FFK, P], BF16, name="hqt")
                    nc.vector.tensor_copy(hqt, ps)
                    return hqt

                def phase3_tile(i, m, hqt):
                    g = i * MT + m
                    stg = st_p.tile([P, DM], F32, name="stg")
                    for d0 in range(0, ND3, 2):
                        nd = min(2, ND3 - d0)
                        ps3 = [
                            psum.tile([P, W], F32, name="ps_main")
                            for _ in range(nd)
                        ]
                        for b in range(FFK):
                            lhsT = hqt[:, b, :]
                            st, sp = (b == 0), (b == FFK - 1)
                            for d in range(nd):
                                nc.tensor.matmul(
                                    ps3[d][:, :W3], lhsT,
                                    wdt_sb[:, b, (d0 + d) * W3 : (d0 + d + 1) * W3],
                                    start=st, stop=sp,
                                )
                        for d in range(nd):
                            nc.vector.tensor_scalar(
                                out=stg[:, (d0 + d) * W3 : (d0 + d + 1) * W3],
                                in0=ps3[d][:, :W3],
                                scalar1=shd_sb[:, g : g + 1], scalar2=None,
                                op0=MULT, op1=BYP,
                            )
                    nc.scalar.dma_start(
                        pout_d[i, m * P : (m + 1) * P, :], stg
                    )

                pending_copies = []

                def chunk_rs(i):
                    # out-copies of finished chunks ride the scalar queue and
                    # are emitted a chunk later so they never wait on the RS
                    while pending_copies:
                        j = pending_copies.pop()
                        nc.scalar.dma_start(
                            out_d[j * TC8 : (j + 1) * TC8, :], rsout_d[j][:]
                        )
                    nc.gpsimd.collective_compute(
                        "ReduceScatter",
                        ADD,
                        replica_groups=rg,
                        ins=[pout_d[i].opt()],
                        outs=[rsout_d[i].opt()],
                    )
                    pending_copies.append(i)

                # Software pipeline over global tile index gg.  Chunk j's
                # quant+P3 tiles are burst-scheduled into the last MT-3
                # iterations of chunk j+1 (2,2,2,1,...), so:
                #  - the absmax-AR(j) result has 3 tiles of slack before
                #    anything consumes it (no FIFO blocks on AR latency),
                #  - P3(j) finishes exactly at chunk j+2's start, where
                #    RS(j) is emitted — it then has a full chunk period on
                #    the gpsimd queue before maxAR(j+2) needs it.
                NTT_ = NC * MT
                # per-iteration assignment: work[off] = list of tile indices m
                assert MT >= 6 or MT == 1
                if MT >= 6:
                    counts = [2] * 3 + [1] * (MT - 6)
                    offs = list(range(3, MT))
                else:  # tiny correctness-only shapes: flat lag schedule
                    counts = [1]
                    offs = [3]
                sched = {}  # iteration gg -> list of (chunk, m)
                for j in range(NC):
                    base = (j + 1) * MT
                    mm = 0
                    for off, cnt in zip(offs, counts):
                        for _ in range(cnt):
                            if mm < MT:
                                sched.setdefault(base + off, []).append((j, mm))
                                mm += 1
                    while mm < MT:  # MT==1 fallback spill
                        sched.setdefault(base + offs[-1] + mm, []).append((j, mm))
                        mm += 1
                p3_last = {}
                for gg2, lst in sched.items():
                    for (j, _m) in lst:
                        p3_last[j] = max(p3_last.get(j, 0), gg2)
                rs_at = {}
                for j in range(NC):
                    rs_at.setdefault(
                        max((j + 2) * MT, p3_last[j] + 1), []
                    ).append(j)
                load_xqc(0, 0)
                last_gg = max(max(sched), max(rs_at)) + 1
                for gg in range(max(NTT_, last_gg) + 1):
                    for j in rs_at.get(gg, ()):
                        chunk_rs(j)
                    if gg < NTT_:
                        i, m = gg // MT, gg % MT
                        if m % MTQ == 0:
                            nq, ni = m // MTQ + 1, i
                            if nq == NPC:
                                nq, ni = 0, i + 1
                            if ni <= NC - 1:
                                load_xqc(ni, nq)
                        phase1_tile(i, m)
                        if m == MT - 1:
                            chunk_absmax_ar(i)
                    gs = gg - (MT + 2)
                    if gs >= 0 and gs % MT == 0 and gs // MT < NC:
                        chunk_scales(gs // MT)
                    for (j, m) in sched.get(gg, ()):
                        hqt = quant_tile(j, m)
                        phase3_tile(j, m, hqt)
                for j in pending_copies:
                    nc.scalar.dma_start(
                        out_d[j * TC8 : (j + 1) * TC8, :], rsout_d[j][:]
                    )

    nc.compile()
    return nc


_CACHE = {}
TRACE = False
LAST_RESULTS = None


def _get_program(TT, DM, FF, NC):
    key = (TT, DM, FF, NC)
    if key not in _CACHE:
        _CACHE[key] = build_program(TT, DM, FF, NC)
    return _CACHE[key]


def kernel(x, w_gate, w_up, w_down):
    from concourse.bass_utils import run_bass_kernel_spmd

    x = np.asarray(x, dtype=np.float32)
    w_gate = np.ascontiguousarray(np.asarray(w_gate, dtype=np.float32))
    w_up = np.ascontiguousarray(np.asarray(w_up, dtype=np.float32))
    w_down = np.ascontiguousarray(np.asarray(w_down, dtype=np.float32))

    B, S, DM = x.shape
    FF = w_gate.shape[0]
    NC = 8
    TT = B * S
    TC = TT // NC
    FFL = FF // NC
    TC8 = TC // NC

    xf = np.ascontiguousarray(x.reshape(TT, DM))
    nc = _get_program(TT, DM, FF, NC)

    in_maps = []
    for c in range(NC):
        in_maps.append(
            {
                "x": np.ascontiguousarray(xf[c * TC : (c + 1) * TC]),
                "wg": np.ascontiguousarray(w_gate[c * FFL : (c + 1) * FFL]),
                "wu": np.ascontiguousarray(w_up[c * FFL : (c + 1) * FFL]),
                "wd": np.ascontiguousarray(w_down[:, c * FFL : (c + 1) * FFL]),
            }
        )

    res = run_bass_kernel_spmd(
        nc, in_maps, core_ids=list(range(NC)), trace=TRACE
    )
    global LAST_RESULTS
    LAST_RESULTS = res
    # core c, chunk i holds tokens i*TC + c*TC8 + [0, TC8)
    out = np.empty((TT, DM), dtype=np.float32)
    for c in range(NC):
        rc = res.results[c]["out_t"].reshape(NC, TC8, DM)
        for i in range(NC):
            t0 = i * TC + c * TC8
            out[t0 : t0 + TC8] = rc[i]
    return out.reshape(B, S, DM)


# revision 4
# speedup vs baseline: 1.0649x; 1.0649x over previous
"""BitNet FFN Trainium2 kernel — 8-core tensor-parallel over d_ff.

Math (forward values of the STE reference):
  wq(w)  = clip(round(w/s), -1, 1) * s,  s = mean(|w|) + EPS        (ternary)
  xq(x)  = round(x/sx) * sx,  sx = max(absmax_row(x), EPS)/127      (int8 range)
  gate = sigmoid(xq @ wq_g.T); up = xq @ wq_u.T; h = gate*up
  out  = hq(h) @ wq_d.T

Sharding: w_gate/w_up rows and w_down columns are sharded 8 ways (ff_local
= d_ff/8 per core), so each core's ternarized weights live in SBUF for the
whole kernel (no weight streaming).  Tokens are processed in 8 chunks of
T/8; every core computes gate/up/h' for ALL tokens against its ff shard.
Each core quantizes its own token shard of x and AllGathers the transposed
bf16 result piece-by-piece (piece-contiguous DRAM layout keeps every DMA
packet large), so chunk i's xqT is exactly AllGather block i.

Per chunk: G/U int matmuls -> sigmoid*U -> local per-token absmax ->
tiny AllReduce(max) for the exact global h scale -> quantize h' ->
down-proj partial matmul (scaled by s_h*s_wd per token, fused into the
PSUM drain) -> f32 ReduceScatter over tokens.  All matmuls are bf16 on
exact integers (|int|<=127 activations, ternary weights) with fp32 PSUM
accumulation, so integer matmuls are exact; scales fold in fp32 outside.

All transposes run on the tensor engine (identity matmul) — DMA
transposes emit 256B packets that flood the DMA queues.  Collectives
serialize on the gpsimd queue and block it while running, so the
schedule keeps that queue empty right before each chunk's absmax-AR and
gives the AR result three tiles of slack before anything consumes it.
"""

import sys

sys.path.insert(0, "/opt/trn_rl_repo")

import contextlib

import numpy as np

import concourse.tile as tile
from concourse import bacc, mybir
from concourse.masks import make_identity

F32 = mybir.dt.float32
BF16 = mybir.dt.bfloat16
ADD = mybir.AluOpType.add
SUB = mybir.AluOpType.subtract
MULT = mybir.AluOpType.mult
MAX = mybir.AluOpType.max
BYP = mybir.AluOpType.bypass
AXX = mybir.AxisListType.X
AFT = mybir.ActivationFunctionType

EPS = 1e-5
CR = 12582912.0  # 1.5*2^23: fp32 RNE round-to-integer magic constant
ALPHA = 1.0986122886681098  # atanh(0.5)/0.5 : tanh(ALPHA*0.5) == 0.5
P = 128
W = 512  # matmul moving free dim (one PSUM bank of f32)


def build_program(TT, DM, FF, NC):
    """TT: total tokens; DM: d_model; FF: d_ff; NC: cores."""
    TC = TT // NC            # tokens per chunk == tokens per x-shard
    MT = TC // P             # token tiles per chunk
    KD = DM // P             # d_model k-blocks
    FFL = FF // NC           # local ff shard
    FFK = FFL // P           # ff k-blocks (phase 3 contraction)
    NFH = max(1, FFL // W)   # ff halves per G/U psum group
    WF = min(W, FFL)
    ND3 = max(1, DM // W)    # dm quarters of the P3 output
    W3 = min(W, DM)
    NPC = min(4, MT)         # xqT AllGather pieces per chunk
    TPQ = TC // NPC
    MTQ = TPQ // P           # token tiles per piece
    NW = float(FF * DM)      # elements per full weight matrix
    TC8 = TC // NC           # RS output rows per chunk per core
    rg = [list(range(NC))]
    assert FFK <= 8 and TPQ % P == 0

    nc = bacc.Bacc(
        "TRN2",
        target_bir_lowering=False,
        debug=False,
        enable_asserts=False,
        num_devices=NC,
    )

    x_d = nc.dram_tensor("x", [TC, DM], F32, kind="ExternalInput")
    wg_d = nc.dram_tensor("wg", [FFL, DM], F32, kind="ExternalInput")
    wu_d = nc.dram_tensor("wu", [FFL, DM], F32, kind="ExternalInput")
    wd_d = nc.dram_tensor("wd", [DM, FFL], F32, kind="ExternalInput")
    out_d = nc.dram_tensor("out_t", [NC * TC8, DM], F32, kind="ExternalOutput")

    with tile.TileContext(nc, num_cores=NC) as tc:
        with contextlib.ExitStack() as outer:
            dram = outer.enter_context(tc.tile_pool(name="dram", bufs=1, space="DRAM"))
            tiny = outer.enter_context(tc.tile_pool(name="tiny", bufs=1))

            # DRAM scratch (xqT is piece-major so every DMA packet is big)
            xqt_sh = [
                dram.tile([P, KD, TPQ], BF16, name=f"xqt_sh{q}") for q in range(NPC)
            ]
            xqt_all = [
                dram.tile([NC, P, KD, TPQ], BF16, name=f"xqt_all{q}",
                          addr_space="Shared")
                for q in range(NPC)
            ]
            sx_sh_d = dram.tile([P, MT], F32)
            sx_all_d = dram.tile([NC, P, MT], F32, addr_space="Shared")
            ws_in = dram.tile([1, 4], F32)
            ws_out = dram.tile([1, 4], F32, addr_space="Shared")
            hmax_in = dram.tile([NC, P, MT], F32)
            hmax_out = [
                dram.tile([P, MT], F32, name=f"hmax_out{i}", addr_space="Shared")
                for i in range(NC)
            ]
            pout_d = dram.tile([NC, TC, DM], F32)
            rsout_d = dram.tile([NC, TC8, DM], F32)

            # persistent small tiles
            ones_row = tiny.tile([1, P], F32)
            nc.vector.memset(ones_row, 1.0)
            ident = tiny.tile([P, P], BF16)
            make_identity(nc, ident)
            sb = tiny.tile([P, 8], F32)   # bcast: bgA,buA,bdA,-,swg,swu,swd,-
            NTT = NC * MT                 # total token tiles
            sx_sb = tiny.tile([P, NTT], F32)
            sxg_sb = tiny.tile([P, NTT], F32)
            sxu_sb = tiny.tile([P, NTT], F32)
            rh_sb = tiny.tile([P, NTT], F32)
            shd_sb = tiny.tile([P, NTT], F32)
            hmax_sb = tiny.tile([P, NTT], F32)

            # persistent ternary weights (bf16, transposed for matmul)
            wgt_sb = tiny.tile([P, KD, FFL], BF16)
            wut_sb = tiny.tile([P, KD, FFL], BF16)
            wdt_sb = tiny.tile([P, FFK, DM], BF16)

            def pe_transpose(src, nblk, dst3, ps_pool):
                """src [P, nblk*P] bf16 -> dst3 [P, nblk, P] (3D slice),
                via PE-transpose through PSUM in groups of <=8 blocks."""
                for h0 in range(0, nblk, 8):
                    nb = min(8, nblk - h0)
                    ps = ps_pool.tile([P, 8, P], BF16, name="ps_tr")
                    for j in range(nb):
                        nc.tensor.transpose(
                            ps[:, j, :], src[:, (h0 + j) * P : (h0 + j + 1) * P],
                            ident,
                        )
                    nc.vector.tensor_copy(
                        dst3[:, h0 : h0 + nb, :], ps[:, :nb, :]
                    )

            # ------------- prologue: x-quant shard + AG, weight scales,
            # ------------- ternarize weights into SBUF
            with contextlib.ExitStack() as pro:
                pspro = pro.enter_context(
                    tc.tile_pool(name="pspro", bufs=3, space="PSUM")
                )
                ps0 = pro.enter_context(
                    tc.tile_pool(name="ps0", bufs=1, space="PSUM")
                )
                xw_p = pro.enter_context(tc.tile_pool(name="xw", bufs=2))
                xtr_p = pro.enter_context(tc.tile_pool(name="xtr", bufs=2))
                s0_p = pro.enter_context(tc.tile_pool(name="s0", bufs=3))
                s0t_p = pro.enter_context(tc.tile_pool(name="s0t", bufs=4))
                wr_p = pro.enter_context(tc.tile_pool(name="wr", bufs=3))
                wt_p = pro.enter_context(tc.tile_pool(name="wt", bufs=2))

                # S0: weight scale sums (|w| over shard, AllReduce later)
                acc3 = tiny.tile([P, 4], F32)
                nc.vector.memset(acc3, 0.0)
                for src, col, rows, cols in (
                    (wg_d, 0, FFL, DM),
                    (wu_d, 1, FFL, DM),
                    (wd_d, 2, DM, FFL),
                ):
                    for r0 in range(0, rows, P):
                        t_in = s0_p.tile([P, DM], F32, name="s0raw")
                        nc.sync.dma_start(t_in[:, :cols], src[r0 : r0 + P, :])
                        t_sum = s0t_p.tile([P, 1], F32, name="s0sum")
                        nc.scalar.activation(
                            out=t_in[:, :cols], in_=t_in[:, :cols],
                            func=AFT.Abs, accum_out=t_sum,
                        )
                        nc.vector.tensor_tensor(
                            out=acc3[:, col : col + 1],
                            in0=acc3[:, col : col + 1],
                            in1=t_sum,
                            op=ADD,
                        )
                ones_col = s0t_p.tile([P, 1], F32, name="ones_col")
                nc.vector.memset(ones_col, 1.0)
                ps_s = ps0.tile([P, W], F32, name="ps_s")
                nc.tensor.matmul(
                    ps_s[:4, :1], acc3[:, :4], ones_col, start=True, stop=True
                )
                sb_s = s0t_p.tile([4, 1], F32, name="sb_s")
                nc.vector.tensor_copy(sb_s, ps_s[:4, :1])
                nc.gpsimd.dma_start(ws_in[0, :4], sb_s[:, 0])
                nc.gpsimd.collective_compute(
                    "AllReduce",
                    ADD,
                    replica_groups=rg,
                    ins=[ws_in[:].opt()],
                    outs=[ws_out[:].opt()],
                )
                sums_row = s0t_p.tile([1, 4], F32, name="sums_row")
                nc.gpsimd.dma_start(sums_row, ws_out[:])
                sw_row = s0t_p.tile([1, 4], F32, name="sw_row")
                nc.vector.tensor_scalar(
                    out=sw_row, in0=sums_row, scalar1=1.0 / NW, scalar2=EPS,
                    op0=MULT, op1=ADD,
                )
                beta_row = s0t_p.tile([1, 4], F32, name="beta_row")
                nc.vector.reciprocal(beta_row, sw_row)
                row8 = s0t_p.tile([1, 8], F32, name="row8")
                nc.vector.tensor_scalar(
                    out=row8[:, 0:4], in0=beta_row, scalar1=ALPHA, scalar2=None,
                    op0=MULT, op1=BYP,
                )
                nc.vector.tensor_copy(row8[:, 4:8], sw_row)
                ps_b = ps0.tile([P, W], F32, name="ps_b")
                nc.tensor.matmul(ps_b[:, :8], ones_row, row8, start=True, stop=True)
                nc.vector.tensor_copy(sb, ps_b[:, :8])

                # X-quant of own token shard -> transposed bf16 pieces -> AG
                sxl = tiny.tile([P, MT], F32)
                xtr_tiles = {}
                for m in range(MT):
                    q, mrel = m // MTQ, m % MTQ
                    if mrel == 0:
                        xtr_tiles[q] = xtr_p.tile([P, KD, TPQ], BF16, name="xtr")
                    xt = xw_p.tile([P, DM], F32, name="xt")
                    nc.sync.dma_start(xt, x_d[m * P : (m + 1) * P, :])
                    amax = s0t_p.tile([P, 1], F32, name="amax")
                    nc.vector.tensor_reduce(
                        amax, xt, axis=AXX, op=MAX, apply_absolute_value=True
                    )
                    nc.vector.tensor_scalar(
                        out=sxl[:, m : m + 1], in0=amax, scalar1=EPS,
                        scalar2=1.0 / 127.0, op0=MAX, op1=MULT,
                    )
                    rxc = s0t_p.tile([P, 1], F32, name="rxc")
                    nc.vector.reciprocal(rxc, sxl[:, m : m + 1])
                    nc.vector.tensor_scalar(
                        out=xt, in0=xt, scalar1=rxc, scalar2=CR, op0=MULT, op1=ADD,
                    )
                    xq = xw_p.tile([P, DM], BF16, name="xq")
                    nc.vector.tensor_scalar(
                        out=xq, in0=xt, scalar1=CR, scalar2=None, op0=SUB, op1=BYP,
                    )
                    for h0 in range(0, KD, 8):
                        nb = min(8, KD - h0)
                        ps = pspro.tile([P, 8, P], BF16, name="ps_tr")
                        for j in range(nb):
                            nc.tensor.transpose(
                                ps[:, j, :], xq[:, (h0 + j) * P : (h0 + j + 1) * P],
                                ident,
                            )
                        nc.vector.tensor_copy(
                            xtr_tiles[q][:, h0 : h0 + nb, mrel * P : (mrel + 1) * P],
                            ps[:, :nb, :],
                        )
                    if mrel == MTQ - 1:
                        nc.sync.dma_start(xqt_sh[q][:], xtr_tiles[q])
                nc.gpsimd.dma_start(sx_sh_d[:], sxl)
                nc.gpsimd.collective_compute(
                    "AllGather",
                    BYP,
                    replica_groups=rg,
                    ins=[sx_sh_d[:].opt()],
                    outs=[sx_all_d[:].opt()],
                )
                # sx readback for all tokens -> per-partition columns
                # (before the big xqt AGs so sigmoid scales aren't gated
                # behind them on the serialized collective queue)
                for i in range(NC):
                    nc.sync.dma_start(
                        sx_sb[:, i * MT : (i + 1) * MT], sx_all_d[i]
                    )
                nc.vector.tensor_scalar(
                    out=sxg_sb, in0=sx_sb, scalar1=sb[:, 4:5], scalar2=None,
                    op0=MULT, op1=BYP,
                )
                nc.vector.tensor_scalar(
                    out=sxu_sb, in0=sx_sb, scalar1=sb[:, 5:6], scalar2=None,
                    op0=MULT, op1=BYP,
                )
                for q in range(NPC):
                    nc.gpsimd.collective_compute(
                        "AllGather",
                        BYP,
                        replica_groups=rg,
                        ins=[xqt_sh[q][:].opt()],
                        outs=[xqt_all[q][:].opt()],
                    )

                # ternarize weights into SBUF (transposed bf16, PE transpose)
                def ternarize(src, beta_col, dst, nblk, rows):
                    cols = nblk * P
                    for r0 in range(0, rows, P):
                        raw = wr_p.tile([P, DM], F32, name="wraw")
                        nc.sync.dma_start(raw[:, :cols], src[r0 : r0 + P, :])
                        nc.scalar.activation(
                            out=raw[:, :cols], in_=raw[:, :cols], func=AFT.Tanh,
                            scale=sb[:, beta_col : beta_col + 1],
                        )
                        tern = wt_p.tile([P, DM], BF16, name="wtern")
                        nc.vector.tensor_scalar(
                            out=tern[:, :cols], in0=raw[:, :cols], scalar1=CR,
                            scalar2=CR, op0=ADD, op1=SUB,
                        )
                        pe_transpose(tern, nblk, dst[:, :, r0 : r0 + P], pspro)

                ternarize(wg_d, 0, wgt_sb, KD, FFL)
                ternarize(wu_d, 1, wut_sb, KD, FFL)
                ternarize(wd_d, 2, wdt_sb, FFK, DM)

            # ------------- main pipeline over 8 token chunks -------------
            with contextlib.ExitStack() as mn:
                psum = mn.enter_context(
                    tc.tile_pool(name="psum", bufs=7, space="PSUM")
                )
                pstr = mn.enter_context(
                    tc.tile_pool(name="pstr", bufs=1, space="PSUM")
                )
                xqc_p = mn.enter_context(tc.tile_pool(name="xqc", bufs=2))
                hp_p = mn.enter_context(tc.tile_pool(name="hp", bufs=MT + 5))
                gt_p = mn.enter_context(tc.tile_pool(name="gt", bufs=2))
                hq_p = mn.enter_context(tc.tile_pool(name="hq", bufs=2))
                hqt_p = mn.enter_context(tc.tile_pool(name="hqt", bufs=4))
                st_p = mn.enter_context(tc.tile_pool(name="st", bufs=2))
                sc_p = mn.enter_context(tc.tile_pool(name="scp", bufs=4))

                hp_tiles = {}
                xqc_tiles = {}

                def load_xqc(i, q):
                    t = xqc_p.tile([P, KD, TPQ], BF16, name="xqc")
                    nc.sync.dma_start(t, xqt_all[q][i])
                    xqc_tiles[(i, q)] = t

                def phase1_tile(i, m):
                    g = i * MT + m
                    xq_t = xqc_tiles[(i, m // MTQ)]
                    trel = (m % MTQ) * P
                    psG = [psum.tile([P, W], F32, name="ps_main") for _ in range(NFH)]
                    psU = [psum.tile([P, W], F32, name="ps_main") for _ in range(NFH)]
                    for k in range(KD):
                        lhsT = xq_t[:, k, trel : trel + P]
                        st, sp = (k == 0), (k == KD - 1)
                        for f in range(NFH):
                            nc.tensor.matmul(
                                psG[f][:, :WF], lhsT,
                                wgt_sb[:, k, f * WF : (f + 1) * WF],
                                start=st, stop=sp,
                            )
                        for f in range(NFH):
                            nc.tensor.matmul(
                                psU[f][:, :WF], lhsT,
                                wut_sb[:, k, f * WF : (f + 1) * WF],
                                start=st, stop=sp,
                            )
                    hp = hp_p.tile([P, FFL], F32, name="hp")
                    for f in range(NFH):
                        gt = gt_p.tile([P, WF], F32, name="gt")
                        nc.scalar.activation(
                            out=gt, in_=psG[f][:, :WF], func=AFT.Sigmoid,
                            scale=sxg_sb[:, g : g + 1],
                        )
                        nc.vector.tensor_tensor(
                            out=hp[:, f * WF : (f + 1) * WF], in0=gt,
                            in1=psU[f][:, :WF], op=MULT,
                        )
                    nc.vector.tensor_reduce(
                        hmax_sb[:, g : g + 1], hp, axis=AXX, op=MAX,
                        apply_absolute_value=True,
                    )
                    hp_tiles[(i, m)] = hp

                def chunk_absmax_ar(i):
                    nc.gpsimd.dma_start(
                        hmax_in[i], hmax_sb[:, i * MT : (i + 1) * MT]
                    )
                    nc.gpsimd.collective_compute(
                        "AllReduce",
                        MAX,
                        replica_groups=rg,
                        ins=[hmax_in[i].opt()],
                        outs=[hmax_out[i][:].opt()],
                    )

                amg_tiles = {}

                def chunk_amg(i):
                    # emitted right after maxAR(i) and BEFORE the next RS on
                    # the gpsimd queue: never blocked by a ReduceScatter, and
                    # keeps the AR wait off the sync queue (xqc prefetch)
                    amg = sc_p.tile([P, MT], F32, name="amg")
                    nc.gpsimd.dma_start(amg, hmax_out[i][:])
                    amg_tiles[i] = amg

                def chunk_scales(i):
                    amg = amg_tiles.pop(i)
                    cs = slice(i * MT, (i + 1) * MT)
                    ah = sc_p.tile([P, MT], F32, name="ah")
                    nc.vector.tensor_tensor(
                        out=ah, in0=amg, in1=sxu_sb[:, cs], op=MULT
                    )
                    sh = sc_p.tile([P, MT], F32, name="sh")
                    nc.vector.tensor_scalar(
                        out=sh, in0=ah, scalar1=EPS, scalar2=1.0 / 127.0,
                        op0=MAX, op1=MULT,
                    )
                    rs_t = sc_p.tile([P, MT], F32, name="rs_t")
                    nc.vector.reciprocal(rs_t, sh)
                    nc.vector.tensor_tensor(
                        out=rh_sb[:, cs], in0=rs_t, in1=sxu_sb[:, cs], op=MULT
                    )
                    nc.vector.tensor_scalar(
                        out=shd_sb[:, cs], in0=sh, scalar1=sb[:, 6:7], scalar2=None,
                        op0=MULT, op1=BYP,
                    )

                def quant_tile(i, m):
                    g = i * MT + m
                    hp = hp_tiles.pop((i, m))
                    nc.scalar.activation(
                        out=hp, in_=hp, func=AFT.Copy,
                        scale=rh_sb[:, g : g + 1], bias=CR,
                    )
                    hq = hq_p.tile([P, FFL], BF16, name="hq")
                    nc.vector.tensor_scalar(
                        out=hq, in0=hp, scalar1=CR, scalar2=None, op0=SUB, op1=BYP,
                    )
                    ps = pstr.tile([P, FFK, P], BF16, name="ps_hqt")
                    for j in range(FFK):
                        nc.tensor.transpose(
                            ps[:, j, :], hq[:, j * P : (j + 1) * P], ident
                        )
                    hqt = hqt_p.tile([P, FFK, P], BF16, name="hqt")
                    nc.vector.tensor_copy(hqt, ps)
                    return hqt

                def phase3_tile(i, m, hqt):
                    g = i * MT + m
                    stg = st_p.tile([P, DM], F32, name="stg")
                    for d0 in range(0, ND3, 2):
                        nd = min(2, ND3 - d0)
                        ps3 = [
                            psum.tile([P, W], F32, name="ps_main")
                            for _ in range(nd)
                        ]
                        for b in range(FFK):
                            lhsT = hqt[:, b, :]
                            st, sp = (b == 0), (b == FFK - 1)
                            for d in range(nd):
                                nc.tensor.matmul(
                                    ps3[d][:, :W3], lhsT,
                                    wdt_sb[:, b, (d0 + d) * W3 : (d0 + d + 1) * W3],
                                    start=st, stop=sp,
                                )
                        for d in range(nd):
                            nc.scalar.activation(
                                out=stg[:, (d0 + d) * W3 : (d0 + d + 1) * W3],
                                in_=ps3[d][:, :W3], func=AFT.Copy,
                                scale=shd_sb[:, g : g + 1],
                            )
                    nc.scalar.dma_start(
                        pout_d[i, m * P : (m + 1) * P, :], stg
                    )

                pending_copies = []

                def chunk_rs(i):
                    # out-copies of finished chunks ride the scalar queue and
                    # are emitted a chunk later so they never wait on the RS
                    while pending_copies:
                        j = pending_copies.pop()
                        nc.scalar.dma_start(
                            out_d[j * TC8 : (j + 1) * TC8, :], rsout_d[j][:]
                        )
                    nc.gpsimd.collective_compute(
                        "ReduceScatter",
                        ADD,
                        replica_groups=rg,
                        ins=[pout_d[i].opt()],
                        outs=[rsout_d[i].opt()],
                    )
                    pending_copies.append(i)

                # Software pipeline over global tile index gg.  Chunk j's
                # quant+P3 tiles are burst-scheduled into the last MT-3
                # iterations of chunk j+1 (2,2,2,1,...), so:
                #  - the absmax-AR(j) result has 3 tiles of slack before
                #    anything consumes it (no FIFO blocks on AR latency),
                #  - P3(j) finishes exactly at chunk j+2's start, where
                #    RS(j) is emitted — it then has a full chunk period on
                #    the gpsimd queue before maxAR(j+2) needs it.
                NTT_ = NC * MT
                # per-iteration assignment: work[off] = list of tile indices m
                assert MT >= 6 or MT == 1
                if MT >= 6:
                    counts = [2] * 3 + [1] * (MT - 6)
                    offs = list(range(3, MT))
                else:  # tiny correctness-only shapes: flat lag schedule
                    counts = [1]
                    offs = [3]
                sched = {}  # iteration gg -> list of (chunk, m)
                for j in range(NC):
                    base = (j + 1) * MT
                    mm = 0
                    for off, cnt in zip(offs, counts):
                        for _ in range(cnt):
                            if mm < MT:
                                sched.setdefault(base + off, []).append((j, mm))
                                mm += 1
                    while mm < MT:  # MT==1 fallback spill
                        sched.setdefault(base + offs[-1] + mm, []).append((j, mm))
                        mm += 1
                p3_last = {}
                for gg2, lst in sched.items():
                    for (j, _m) in lst:
                        p3_last[j] = max(p3_last.get(j, 0), gg2)
                rs_at = {}
                for j in range(NC):
                    rs_at.setdefault(
                        max((j + 2) * MT, p3_last[j] + 1), []
                    ).append(j)
                load_xqc(0, 0)
                last_gg = max(max(sched), max(rs_at)) + 1
                for gg in range(max(NTT_, last_gg) + 1):
                    if gg % MT == 0 and 1 <= gg // MT <= NC:
                        chunk_amg(gg // MT - 1)
                    for j in rs_at.get(gg, ()):
                        chunk_rs(j)
                    if gg < NTT_:
                        i, m = gg // MT, gg % MT
                        if m % MTQ == 0:
                            nq, ni = m // MTQ + 1, i
                            if nq == NPC:
                                nq, ni = 0, i + 1
                            if ni <= NC - 1:
                                load_xqc(ni, nq)
                        phase1_tile(i, m)
                        if m == MT - 1:
                            chunk_absmax_ar(i)
                    gs = gg - (MT + 2)
                    if gs >= 0 and gs % MT == 0 and gs // MT < NC:
                        chunk_scales(gs // MT)
                    for (j, m) in sched.get(gg, ()):
                        hqt = quant_tile(j, m)
                        phase3_tile(j, m, hqt)
                for j in pending_copies:
                    nc.scalar.dma_start(
                        out_d[j * TC8 : (j + 1) * TC8, :], rsout_d[j][:]
                    )

    nc.compile()
    return nc


_CACHE = {}
TRACE = False
LAST_RESULTS = None


def _get_program(TT, DM, FF, NC):
    key = (TT, DM, FF, NC)
    if key not in _CACHE:
        _CACHE[key] = build_program(TT, DM, FF, NC)
    return _CACHE[key]


def kernel(x, w_gate, w_up, w_down):
    from concourse.bass_utils import run_bass_kernel_spmd

    x = np.asarray(x, dtype=np.float32)
    w_gate = np.ascontiguousarray(np.asarray(w_gate, dtype=np.float32))
    w_up = np.ascontiguousarray(np.asarray(w_up, dtype=np.float32))
    w_down = np.ascontiguousarray(np.asarray(w_down, dtype=np.float32))

    B, S, DM = x.shape
    FF = w_gate.shape[0]
    NC = 8
    TT = B * S
    TC = TT // NC
    FFL = FF // NC
    TC8 = TC // NC

    xf = np.ascontiguousarray(x.reshape(TT, DM))
    nc = _get_program(TT, DM, FF, NC)

    in_maps = []
    for c in range(NC):
        in_maps.append(
            {
                "x": np.ascontiguousarray(xf[c * TC : (c + 1) * TC]),
                "wg": np.ascontiguousarray(w_gate[c * FFL : (c + 1) * FFL]),
                "wu": np.ascontiguousarray(w_up[c * FFL : (c + 1) * FFL]),
                "wd": np.ascontiguousarray(w_down[:, c * FFL : (c + 1) * FFL]),
            }
        )

    res = run_bass_kernel_spmd(
        nc, in_maps, core_ids=list(range(NC)), trace=TRACE
    )
    global LAST_RESULTS
    LAST_RESULTS = res
    # core c, chunk i holds tokens i*TC + c*TC8 + [0, TC8)
    out = np.empty((TT, DM), dtype=np.float32)
    for c in range(NC):
        rc = res.results[c]["out_t"].reshape(NC, TC8, DM)
        for i in range(NC):
            t0 = i * TC + c * TC8
            out[t0 : t0 + TC8] = rc[i]
    return out.reshape(B, S, DM)


# revision 5
# speedup vs baseline: 1.0685x; 1.0033x over previous
"""BitNet FFN Trainium2 kernel — 8-core tensor-parallel over d_ff.

Math (forward values of the STE reference):
  wq(w)  = clip(round(w/s), -1, 1) * s,  s = mean(|w|) + EPS        (ternary)
  xq(x)  = round(x/sx) * sx,  sx = max(absmax_row(x), EPS)/127      (int8 range)
  gate = sigmoid(xq @ wq_g.T); up = xq @ wq_u.T; h = gate*up
  out  = hq(h) @ wq_d.T

Sharding: w_gate/w_up rows and w_down columns are sharded 8 ways (ff_local
= d_ff/8 per core), so each core's ternarized weights live in SBUF for the
whole kernel (no weight streaming).  Tokens are processed in 8 chunks of
T/8; every core computes gate/up/h' for ALL tokens against its ff shard.
Each core quantizes its own token shard of x and AllGathers the transposed
bf16 result piece-by-piece (piece-contiguous DRAM layout keeps every DMA
packet large), so chunk i's xqT is exactly AllGather block i.

Per chunk: G/U int matmuls -> sigmoid*U -> local per-token absmax ->
tiny AllReduce(max) for the exact global h scale -> quantize h' ->
down-proj partial matmul (scaled by s_h*s_wd per token, fused into the
PSUM drain) -> f32 ReduceScatter over tokens.  All matmuls are bf16 on
exact integers (|int|<=127 activations, ternary weights) with fp32 PSUM
accumulation, so integer matmuls are exact; scales fold in fp32 outside.

All transposes run on the tensor engine (identity matmul) — DMA
transposes emit 256B packets that flood the DMA queues.  Collectives
serialize on the gpsimd queue and block it while running, so the
schedule keeps that queue empty right before each chunk's absmax-AR and
gives the AR result three tiles of slack before anything consumes it.
"""

import sys

sys.path.insert(0, "/opt/trn_rl_repo")

import contextlib

import numpy as np

import concourse.tile as tile
from concourse import bacc, mybir
from concourse.masks import make_identity

F32 = mybir.dt.float32
BF16 = mybir.dt.bfloat16
ADD = mybir.AluOpType.add
SUB = mybir.AluOpType.subtract
MULT = mybir.AluOpType.mult
MAX = mybir.AluOpType.max
BYP = mybir.AluOpType.bypass
AXX = mybir.AxisListType.X
AFT = mybir.ActivationFunctionType

EPS = 1e-5
CR = 12582912.0  # 1.5*2^23: fp32 RNE round-to-integer magic constant
ALPHA = 1.0986122886681098  # atanh(0.5)/0.5 : tanh(ALPHA*0.5) == 0.5
P = 128
W = 512  # matmul moving free dim (one PSUM bank of f32)


def build_program(TT, DM, FF, NC):
    """TT: total tokens; DM: d_model; FF: d_ff; NC: cores."""
    TC = TT // NC            # tokens per chunk == tokens per x-shard
    MT = TC // P             # token tiles per chunk
    KD = DM // P             # d_model k-blocks
    FFL = FF // NC           # local ff shard
    FFK = FFL // P           # ff k-blocks (phase 3 contraction)
    NFH = max(1, FFL // W)   # ff halves per G/U psum group
    WF = min(W, FFL)
    ND3 = max(1, DM // W)    # dm quarters of the P3 output
    W3 = min(W, DM)
    NPC = min(4, MT)         # xqT AllGather pieces per chunk
    TPQ = TC // NPC
    MTQ = TPQ // P           # token tiles per piece
    NW = float(FF * DM)      # elements per full weight matrix
    TC8 = TC // NC           # RS output rows per chunk per core
    rg = [list(range(NC))]
    assert FFK <= 8 and TPQ % P == 0

    nc = bacc.Bacc(
        "TRN2",
        target_bir_lowering=False,
        debug=False,
        enable_asserts=False,
        num_devices=NC,
    )

    x_d = nc.dram_tensor("x", [TC, DM], F32, kind="ExternalInput")
    wg_d = nc.dram_tensor("wg", [FFL, DM], F32, kind="ExternalInput")
    wu_d = nc.dram_tensor("wu", [FFL, DM], F32, kind="ExternalInput")
    wd_d = nc.dram_tensor("wd", [DM, FFL], F32, kind="ExternalInput")
    out_d = nc.dram_tensor("out_t", [NC * TC8, DM], F32, kind="ExternalOutput")

    with tile.TileContext(nc, num_cores=NC) as tc:
        with contextlib.ExitStack() as outer:
            dram = outer.enter_context(tc.tile_pool(name="dram", bufs=1, space="DRAM"))
            tiny = outer.enter_context(tc.tile_pool(name="tiny", bufs=1))

            # DRAM scratch (xqT is piece-major so every DMA packet is big)
            xqt_sh = [
                dram.tile([P, KD, TPQ], BF16, name=f"xqt_sh{q}") for q in range(NPC)
            ]
            xqt_all = [
                dram.tile([NC, P, KD, TPQ], BF16, name=f"xqt_all{q}",
                          addr_space="Shared")
                for q in range(NPC)
            ]
            sx_sh_d = dram.tile([P, MT], F32)
            sx_all_d = dram.tile([NC, P, MT], F32, addr_space="Shared")
            ws_in = dram.tile([1, 4], F32)
            ws_out = dram.tile([1, 4], F32, addr_space="Shared")
            hmax_in = dram.tile([NC, P, MT], F32)
            hmax_out = [
                dram.tile([P, MT], F32, name=f"hmax_out{i}", addr_space="Shared")
                for i in range(NC)
            ]
            pout_d = dram.tile([NC, TC, DM], BF16)
            rsout_d = dram.tile([NC, TC8, DM], BF16)

            # persistent small tiles
            ones_row = tiny.tile([1, P], F32)
            nc.vector.memset(ones_row, 1.0)
            ident = tiny.tile([P, P], BF16)
            make_identity(nc, ident)
            sb = tiny.tile([P, 8], F32)   # bcast: bgA,buA,bdA,-,swg,swu,swd,-
            NTT = NC * MT                 # total token tiles
            sx_sb = tiny.tile([P, NTT], F32)
            sxg_sb = tiny.tile([P, NTT], F32)
            sxu_sb = tiny.tile([P, NTT], F32)
            rh_sb = tiny.tile([P, NTT], F32)
            shd_sb = tiny.tile([P, NTT], F32)
            hmax_sb = tiny.tile([P, NTT], F32)

            # persistent ternary weights (bf16, transposed for matmul)
            wgt_sb = tiny.tile([P, KD, FFL], BF16)
            wut_sb = tiny.tile([P, KD, FFL], BF16)
            wdt_sb = tiny.tile([P, FFK, DM], BF16)

            def pe_transpose(src, nblk, dst3, ps_pool):
                """src [P, nblk*P] bf16 -> dst3 [P, nblk, P] (3D slice),
                via PE-transpose through PSUM in groups of <=8 blocks."""
                for h0 in range(0, nblk, 8):
                    nb = min(8, nblk - h0)
                    ps = ps_pool.tile([P, 8, P], BF16, name="ps_tr")
                    for j in range(nb):
                        nc.tensor.transpose(
                            ps[:, j, :], src[:, (h0 + j) * P : (h0 + j + 1) * P],
                            ident,
                        )
                    nc.vector.tensor_copy(
                        dst3[:, h0 : h0 + nb, :], ps[:, :nb, :]
                    )

            # ------------- prologue: x-quant shard + AG, weight scales,
            # ------------- ternarize weights into SBUF
            with contextlib.ExitStack() as pro:
                pspro = pro.enter_context(
                    tc.tile_pool(name="pspro", bufs=3, space="PSUM")
                )
                ps0 = pro.enter_context(
                    tc.tile_pool(name="ps0", bufs=1, space="PSUM")
                )
                xw_p = pro.enter_context(tc.tile_pool(name="xw", bufs=2))
                xtr_p = pro.enter_context(tc.tile_pool(name="xtr", bufs=2))
                s0_p = pro.enter_context(tc.tile_pool(name="s0", bufs=3))
                s0t_p = pro.enter_context(tc.tile_pool(name="s0t", bufs=4))
                wr_p = pro.enter_context(tc.tile_pool(name="wr", bufs=3))
                wt_p = pro.enter_context(tc.tile_pool(name="wt", bufs=2))

                # S0: weight scale sums (|w| over shard, AllReduce later)
                acc3 = tiny.tile([P, 4], F32)
                nc.vector.memset(acc3, 0.0)
                for src, col, rows, cols in (
                    (wg_d, 0, FFL, DM),
                    (wu_d, 1, FFL, DM),
                    (wd_d, 2, DM, FFL),
                ):
                    for r0 in range(0, rows, P):
                        t_in = s0_p.tile([P, DM], F32, name="s0raw")
                        nc.sync.dma_start(t_in[:, :cols], src[r0 : r0 + P, :])
                        t_sum = s0t_p.tile([P, 1], F32, name="s0sum")
                        nc.scalar.activation(
                            out=t_in[:, :cols], in_=t_in[:, :cols],
                            func=AFT.Abs, accum_out=t_sum,
                        )
                        nc.vector.tensor_tensor(
                            out=acc3[:, col : col + 1],
                            in0=acc3[:, col : col + 1],
                            in1=t_sum,
                            op=ADD,
                        )
                ones_col = s0t_p.tile([P, 1], F32, name="ones_col")
                nc.vector.memset(ones_col, 1.0)
                ps_s = ps0.tile([P, W], F32, name="ps_s")
                nc.tensor.matmul(
                    ps_s[:4, :1], acc3[:, :4], ones_col, start=True, stop=True
                )
                sb_s = s0t_p.tile([4, 1], F32, name="sb_s")
                nc.vector.tensor_copy(sb_s, ps_s[:4, :1])
                nc.gpsimd.dma_start(ws_in[0, :4], sb_s[:, 0])
                nc.gpsimd.collective_compute(
                    "AllReduce",
                    ADD,
                    replica_groups=rg,
                    ins=[ws_in[:].opt()],
                    outs=[ws_out[:].opt()],
                )
                sums_row = s0t_p.tile([1, 4], F32, name="sums_row")
                nc.gpsimd.dma_start(sums_row, ws_out[:])
                sw_row = s0t_p.tile([1, 4], F32, name="sw_row")
                nc.vector.tensor_scalar(
                    out=sw_row, in0=sums_row, scalar1=1.0 / NW, scalar2=EPS,
                    op0=MULT, op1=ADD,
                )
                beta_row = s0t_p.tile([1, 4], F32, name="beta_row")
                nc.vector.reciprocal(beta_row, sw_row)
                row8 = s0t_p.tile([1, 8], F32, name="row8")
                nc.vector.tensor_scalar(
                    out=row8[:, 0:4], in0=beta_row, scalar1=ALPHA, scalar2=None,
                    op0=MULT, op1=BYP,
                )
                nc.vector.tensor_copy(row8[:, 4:8], sw_row)
                ps_b = ps0.tile([P, W], F32, name="ps_b")
                nc.tensor.matmul(ps_b[:, :8], ones_row, row8, start=True, stop=True)
                nc.vector.tensor_copy(sb, ps_b[:, :8])

                # X-quant of own token shard -> transposed bf16 pieces -> AG
                sxl = tiny.tile([P, MT], F32)
                xtr_tiles = {}
                for m in range(MT):
                    q, mrel = m // MTQ, m % MTQ
                    if mrel == 0:
                        xtr_tiles[q] = xtr_p.tile([P, KD, TPQ], BF16, name="xtr")
                    xt = xw_p.tile([P, DM], F32, name="xt")
                    nc.sync.dma_start(xt, x_d[m * P : (m + 1) * P, :])
                    amax = s0t_p.tile([P, 1], F32, name="amax")
                    nc.vector.tensor_reduce(
                        amax, xt, axis=AXX, op=MAX, apply_absolute_value=True
                    )
                    nc.vector.tensor_scalar(
                        out=sxl[:, m : m + 1], in0=amax, scalar1=EPS,
                        scalar2=1.0 / 127.0, op0=MAX, op1=MULT,
                    )
                    rxc = s0t_p.tile([P, 1], F32, name="rxc")
                    nc.vector.reciprocal(rxc, sxl[:, m : m + 1])
                    nc.vector.tensor_scalar(
                        out=xt, in0=xt, scalar1=rxc, scalar2=CR, op0=MULT, op1=ADD,
                    )
                    xq = xw_p.tile([P, DM], BF16, name="xq")
                    nc.vector.tensor_scalar(
                        out=xq, in0=xt, scalar1=CR, scalar2=None, op0=SUB, op1=BYP,
                    )
                    for h0 in range(0, KD, 8):
                        nb = min(8, KD - h0)
                        ps = pspro.tile([P, 8, P], BF16, name="ps_tr")
                        for j in range(nb):
                            nc.tensor.transpose(
                                ps[:, j, :], xq[:, (h0 + j) * P : (h0 + j + 1) * P],
                                ident,
                            )
                        nc.vector.tensor_copy(
                            xtr_tiles[q][:, h0 : h0 + nb, mrel * P : (mrel + 1) * P],
                            ps[:, :nb, :],
                        )
                    if mrel == MTQ - 1:
                        nc.sync.dma_start(xqt_sh[q][:], xtr_tiles[q])
                nc.gpsimd.dma_start(sx_sh_d[:], sxl)
                nc.gpsimd.collective_compute(
                    "AllGather",
                    BYP,
                    replica_groups=rg,
                    ins=[sx_sh_d[:].opt()],
                    outs=[sx_all_d[:].opt()],
                )
                # sx readback for all tokens -> per-partition columns
                # (before the big xqt AGs so sigmoid scales aren't gated
                # behind them on the serialized collective queue)
                for i in range(NC):
                    nc.sync.dma_start(
                        sx_sb[:, i * MT : (i + 1) * MT], sx_all_d[i]
                    )
                nc.vector.tensor_scalar(
                    out=sxg_sb, in0=sx_sb, scalar1=sb[:, 4:5], scalar2=None,
                    op0=MULT, op1=BYP,
                )
                nc.vector.tensor_scalar(
                    out=sxu_sb, in0=sx_sb, scalar1=sb[:, 5:6], scalar2=None,
                    op0=MULT, op1=BYP,
                )
                for q in range(NPC):
                    nc.gpsimd.collective_compute(
                        "AllGather",
                        BYP,
                        replica_groups=rg,
                        ins=[xqt_sh[q][:].opt()],
                        outs=[xqt_all[q][:].opt()],
                    )

                # ternarize weights into SBUF (transposed bf16, PE transpose)
                def ternarize(src, beta_col, dst, nblk, rows):
                    cols = nblk * P
                    for r0 in range(0, rows, P):
                        raw = wr_p.tile([P, DM], F32, name="wraw")
                        nc.sync.dma_start(raw[:, :cols], src[r0 : r0 + P, :])
                        nc.scalar.activation(
                            out=raw[:, :cols], in_=raw[:, :cols], func=AFT.Tanh,
                            scale=sb[:, beta_col : beta_col + 1],
                        )
                        tern = wt_p.tile([P, DM], BF16, name="wtern")
                        nc.vector.tensor_scalar(
                            out=tern[:, :cols], in0=raw[:, :cols], scalar1=CR,
                            scalar2=CR, op0=ADD, op1=SUB,
                        )
                        pe_transpose(tern, nblk, dst[:, :, r0 : r0 + P], pspro)

                ternarize(wg_d, 0, wgt_sb, KD, FFL)
                ternarize(wu_d, 1, wut_sb, KD, FFL)
                ternarize(wd_d, 2, wdt_sb, FFK, DM)

            # ------------- main pipeline over 8 token chunks -------------
            with contextlib.ExitStack() as mn:
                psum = mn.enter_context(
                    tc.tile_pool(name="psum", bufs=7, space="PSUM")
                )
                pstr = mn.enter_context(
                    tc.tile_pool(name="pstr", bufs=1, space="PSUM")
                )
                xqc_p = mn.enter_context(tc.tile_pool(name="xqc", bufs=2))
                hp_p = mn.enter_context(tc.tile_pool(name="hp", bufs=MT + 5))
                gt_p = mn.enter_context(tc.tile_pool(name="gt", bufs=2))
                hq_p = mn.enter_context(tc.tile_pool(name="hq", bufs=2))
                hqt_p = mn.enter_context(tc.tile_pool(name="hqt", bufs=4))
                st_p = mn.enter_context(tc.tile_pool(name="st", bufs=2))
                sc_p = mn.enter_context(tc.tile_pool(name="scp", bufs=4))

                hp_tiles = {}
                xqc_tiles = {}

                def load_xqc(i, q):
                    t = xqc_p.tile([P, KD, TPQ], BF16, name="xqc")
                    nc.sync.dma_start(t, xqt_all[q][i])
                    xqc_tiles[(i, q)] = t

                def phase1_tile(i, m):
                    g = i * MT + m
                    xq_t = xqc_tiles[(i, m // MTQ)]
                    trel = (m % MTQ) * P
                    psG = [psum.tile([P, W], F32, name="ps_main") for _ in range(NFH)]
                    psU = [psum.tile([P, W], F32, name="ps_main") for _ in range(NFH)]
                    for k in range(KD):
                        lhsT = xq_t[:, k, trel : trel + P]
                        st, sp = (k == 0), (k == KD - 1)
                        for f in range(NFH):
                            nc.tensor.matmul(
                                psG[f][:, :WF], lhsT,
                                wgt_sb[:, k, f * WF : (f + 1) * WF],
                                start=st, stop=sp,
                            )
                        for f in range(NFH):
                            nc.tensor.matmul(
                                psU[f][:, :WF], lhsT,
                                wut_sb[:, k, f * WF : (f + 1) * WF],
                                start=st, stop=sp,
                            )
                    hp = hp_p.tile([P, FFL], F32, name="hp")
                    for f in range(NFH):
                        gt = gt_p.tile([P, WF], F32, name="gt")
                        nc.scalar.activation(
                            out=gt, in_=psG[f][:, :WF], func=AFT.Sigmoid,
                            scale=sxg_sb[:, g : g + 1],
                        )
                        nc.vector.tensor_tensor(
                            out=hp[:, f * WF : (f + 1) * WF], in0=gt,
                            in1=psU[f][:, :WF], op=MULT,
                        )
                    nc.vector.tensor_reduce(
                        hmax_sb[:, g : g + 1], hp, axis=AXX, op=MAX,
                        apply_absolute_value=True,
                    )
                    hp_tiles[(i, m)] = hp

                def chunk_absmax_ar(i):
                    nc.gpsimd.dma_start(
                        hmax_in[i], hmax_sb[:, i * MT : (i + 1) * MT]
                    )
                    nc.gpsimd.collective_compute(
                        "AllReduce",
                        MAX,
                        replica_groups=rg,
                        ins=[hmax_in[i].opt()],
                        outs=[hmax_out[i][:].opt()],
                    )

                amg_tiles = {}

                def chunk_amg(i):
                    # emitted right after maxAR(i) and BEFORE the next RS on
                    # the gpsimd queue: never blocked by a ReduceScatter, and
                    # keeps the AR wait off the sync queue (xqc prefetch)
                    amg = sc_p.tile([P, MT], F32, name="amg")
                    nc.gpsimd.dma_start(amg, hmax_out[i][:])
                    amg_tiles[i] = amg

                def chunk_scales(i):
                    amg = amg_tiles.pop(i)
                    cs = slice(i * MT, (i + 1) * MT)
                    ah = sc_p.tile([P, MT], F32, name="ah")
                    nc.vector.tensor_tensor(
                        out=ah, in0=amg, in1=sxu_sb[:, cs], op=MULT
                    )
                    sh = sc_p.tile([P, MT], F32, name="sh")
                    nc.vector.tensor_scalar(
                        out=sh, in0=ah, scalar1=EPS, scalar2=1.0 / 127.0,
                        op0=MAX, op1=MULT,
                    )
                    rs_t = sc_p.tile([P, MT], F32, name="rs_t")
                    nc.vector.reciprocal(rs_t, sh)
                    nc.vector.tensor_tensor(
                        out=rh_sb[:, cs], in0=rs_t, in1=sxu_sb[:, cs], op=MULT
                    )
                    nc.vector.tensor_scalar(
                        out=shd_sb[:, cs], in0=sh, scalar1=sb[:, 6:7], scalar2=None,
                        op0=MULT, op1=BYP,
                    )

                def quant_tile(i, m):
                    g = i * MT + m
                    hp = hp_tiles.pop((i, m))
                    nc.scalar.activation(
                        out=hp, in_=hp, func=AFT.Copy,
                        scale=rh_sb[:, g : g + 1], bias=CR,
                    )
                    hq = hq_p.tile([P, FFL], BF16, name="hq")
                    nc.vector.tensor_scalar(
                        out=hq, in0=hp, scalar1=CR, scalar2=None, op0=SUB, op1=BYP,
                    )
                    ps = pstr.tile([P, FFK, P], BF16, name="ps_hqt")
                    for j in range(FFK):
                        nc.tensor.transpose(
                            ps[:, j, :], hq[:, j * P : (j + 1) * P], ident
                        )
                    hqt = hqt_p.tile([P, FFK, P], BF16, name="hqt")
                    nc.vector.tensor_copy(hqt, ps)
                    return hqt

                def phase3_tile(i, m, hqt):
                    g = i * MT + m
                    stg = st_p.tile([P, DM], BF16, name="stg")
                    for d0 in range(0, ND3, 2):
                        nd = min(2, ND3 - d0)
                        ps3 = [
                            psum.tile([P, W], F32, name="ps_main")
                            for _ in range(nd)
                        ]
                        for b in range(FFK):
                            lhsT = hqt[:, b, :]
                            st, sp = (b == 0), (b == FFK - 1)
                            for d in range(nd):
                                nc.tensor.matmul(
                                    ps3[d][:, :W3], lhsT,
                                    wdt_sb[:, b, (d0 + d) * W3 : (d0 + d + 1) * W3],
                                    start=st, stop=sp,
                                )
                        for d in range(nd):
                            nc.scalar.activation(
                                out=stg[:, (d0 + d) * W3 : (d0 + d + 1) * W3],
                                in_=ps3[d][:, :W3], func=AFT.Copy,
                                scale=shd_sb[:, g : g + 1],
                            )
                    nc.scalar.dma_start(
                        pout_d[i, m * P : (m + 1) * P, :], stg
                    )

                pending_copies = []
                rsin_p = mn.enter_context(tc.tile_pool(name="rsin", bufs=1))
                cvt_p = mn.enter_context(tc.tile_pool(name="cvt", bufs=1))

                def emit_out_copy(j):
                    rsin = rsin_p.tile([TC8, DM], BF16, name="rsin")
                    nc.scalar.dma_start(rsin, rsout_d[j][:])
                    cvt = cvt_p.tile([TC8, DM], F32, name="cvt")
                    nc.scalar.activation(out=cvt, in_=rsin, func=AFT.Copy)
                    nc.scalar.dma_start(out_d[j * TC8 : (j + 1) * TC8, :], cvt)

                def chunk_rs(i):
                    # out-copies of finished chunks ride the scalar queue and
                    # are emitted a chunk later so they never wait on the RS
                    while pending_copies:
                        emit_out_copy(pending_copies.pop())
                    nc.gpsimd.collective_compute(
                        "ReduceScatter",
                        ADD,
                        replica_groups=rg,
                        ins=[pout_d[i].opt()],
                        outs=[rsout_d[i].opt()],
                    )
                    pending_copies.append(i)

                # Software pipeline over global tile index gg.  Chunk j's
                # quant+P3 tiles are burst-scheduled into the last MT-3
                # iterations of chunk j+1 (2,2,2,1,...), so:
                #  - the absmax-AR(j) result has 3 tiles of slack before
                #    anything consumes it (no FIFO blocks on AR latency),
                #  - P3(j) finishes exactly at chunk j+2's start, where
                #    RS(j) is emitted — it then has a full chunk period on
                #    the gpsimd queue before maxAR(j+2) needs it.
                NTT_ = NC * MT
                # per-iteration assignment: work[off] = list of tile indices m
                assert MT >= 6 or MT == 1
                if MT >= 6:
                    counts = [2] * 3 + [1] * (MT - 6)
                    offs = list(range(3, MT))
                else:  # tiny correctness-only shapes: flat lag schedule
                    counts = [1]
                    offs = [3]
                sched = {}  # iteration gg -> list of (chunk, m)
                for j in range(NC):
                    base = (j + 1) * MT
                    mm = 0
                    for off, cnt in zip(offs, counts):
                        for _ in range(cnt):
                            if mm < MT:
                                sched.setdefault(base + off, []).append((j, mm))
                                mm += 1
                    while mm < MT:  # MT==1 fallback spill
                        sched.setdefault(base + offs[-1] + mm, []).append((j, mm))
                        mm += 1
                p3_last = {}
                for gg2, lst in sched.items():
                    for (j, _m) in lst:
                        p3_last[j] = max(p3_last.get(j, 0), gg2)
                rs_at = {}
                for j in range(NC):
                    rs_at.setdefault(
                        max((j + 2) * MT, p3_last[j] + 1), []
                    ).append(j)
                load_xqc(0, 0)
                last_gg = max(max(sched), max(rs_at)) + 1
                for gg in range(max(NTT_, last_gg) + 1):
                    if gg % MT == 0 and 1 <= gg // MT <= NC:
                        chunk_amg(gg // MT - 1)
                    for j in rs_at.get(gg, ()):
                        chunk_rs(j)
                    if gg < NTT_:
                        i, m = gg // MT, gg % MT
                        if m % MTQ == 0:
                            nq, ni = m // MTQ + 1, i
                            if nq == NPC:
                                nq, ni = 0, i + 1
                            if ni <= NC - 1:
                                load_xqc(ni, nq)
                        phase1_tile(i, m)
                        if m == MT - 1:
                            chunk_absmax_ar(i)
                    gs = gg - (MT + 2)
                    if gs >= 0 and gs % MT == 0 and gs // MT < NC:
                        chunk_scales(gs // MT)
                    for (j, m) in sched.get(gg, ()):
                        hqt = quant_tile(j, m)
                        phase3_tile(j, m, hqt)
                for j in pending_copies:
                    emit_out_copy(j)

    nc.compile()
    return nc


_CACHE = {}
TRACE = False
LAST_RESULTS = None


def _get_program(TT, DM, FF, NC):
    key = (TT, DM, FF, NC)
    if key not in _CACHE:
        _CACHE[key] = build_program(TT, DM, FF, NC)
    return _CACHE[key]


def kernel(x, w_gate, w_up, w_down):
    from concourse.bass_utils import run_bass_kernel_spmd

    x = np.asarray(x, dtype=np.float32)
    w_gate = np.ascontiguousarray(np.asarray(w_gate, dtype=np.float32))
    w_up = np.ascontiguousarray(np.asarray(w_up, dtype=np.float32))
    w_down = np.ascontiguousarray(np.asarray(w_down, dtype=np.float32))

    B, S, DM = x.shape
    FF = w_gate.shape[0]
    NC = 8
    TT = B * S
    TC = TT // NC
    FFL = FF // NC
    TC8 = TC // NC

    xf = np.ascontiguousarray(x.reshape(TT, DM))
    nc = _get_program(TT, DM, FF, NC)

    in_maps = []
    for c in range(NC):
        in_maps.append(
            {
                "x": np.ascontiguousarray(xf[c * TC : (c + 1) * TC]),
                "wg": np.ascontiguousarray(w_gate[c * FFL : (c + 1) * FFL]),
                "wu": np.ascontiguousarray(w_up[c * FFL : (c + 1) * FFL]),
                "wd": np.ascontiguousarray(w_down[:, c * FFL : (c + 1) * FFL]),
            }
        )

    res = run_bass_kernel_spmd(
        nc, in_maps, core_ids=list(range(NC)), trace=TRACE
    )
    global LAST_RESULTS
    LAST_RESULTS = res
    # core c, chunk i holds tokens i*TC + c*TC8 + [0, TC8)
    out = np.empty((TT, DM), dtype=np.float32)
    for c in range(NC):
        rc = res.results[c]["out_t"].reshape(NC, TC8, DM)
        for i in range(NC):
            t0 = i * TC + c * TC8
            out[t0 : t0 + TC8] = rc[i]
    return out.reshape(B, S, DM)


# revision 6
# speedup vs baseline: 1.1238x; 1.0518x over previous
"""BitNet FFN Trainium2 kernel — 8-core tensor-parallel over d_ff.

Math (forward values of the STE reference):
  wq(w)  = clip(round(w/s), -1, 1) * s,  s = mean(|w|) + EPS        (ternary)
  xq(x)  = round(x/sx) * sx,  sx = max(absmax_row(x), EPS)/127      (int8 range)
  gate = sigmoid(xq @ wq_g.T); up = xq @ wq_u.T; h = gate*up
  out  = hq(h) @ wq_d.T

Sharding: w_gate/w_up rows and w_down columns are sharded 8 ways (ff_local
= d_ff/8 per core), so each core's ternarized weights live in SBUF for the
whole kernel (no weight streaming).  Tokens are processed in 8 chunks of
T/8; every core computes gate/up/h' for ALL tokens against its ff shard.
Each core quantizes its own token shard of x and AllGathers the transposed
bf16 result piece-by-piece (piece-contiguous DRAM layout keeps every DMA
packet large), so chunk i's xqT is exactly AllGather block i.

Per chunk: G/U int matmuls -> sigmoid*U -> local per-token absmax ->
tiny AllReduce(max) for the exact global h scale -> quantize h' ->
down-proj partial matmul (scaled by s_h*s_wd per token, fused into the
PSUM drain) -> f32 ReduceScatter over tokens.  All matmuls are bf16 on
exact integers (|int|<=127 activations, ternary weights) with fp32 PSUM
accumulation, so integer matmuls are exact; scales fold in fp32 outside.

All transposes run on the tensor engine (identity matmul) — DMA
transposes emit 256B packets that flood the DMA queues.  Collectives
serialize on the gpsimd queue and block it while running, so the
schedule keeps that queue empty right before each chunk's absmax-AR and
gives the AR result three tiles of slack before anything consumes it.
"""

import sys

sys.path.insert(0, "/opt/trn_rl_repo")

import contextlib

import numpy as np

import concourse.tile as tile
from concourse import bacc, mybir
from concourse.masks import make_identity

F32 = mybir.dt.float32
BF16 = mybir.dt.bfloat16
ADD = mybir.AluOpType.add
SUB = mybir.AluOpType.subtract
MULT = mybir.AluOpType.mult
MAX = mybir.AluOpType.max
BYP = mybir.AluOpType.bypass
AXX = mybir.AxisListType.X
AFT = mybir.ActivationFunctionType

EPS = 1e-5
CR = 12582912.0  # 1.5*2^23: fp32 RNE round-to-integer magic constant
ALPHA = 1.0986122886681098  # atanh(0.5)/0.5 : tanh(ALPHA*0.5) == 0.5
P = 128
W = 512  # matmul moving free dim (one PSUM bank of f32)


def build_program(TT, DM, FF, NC):
    """TT: total tokens; DM: d_model; FF: d_ff; NC: cores."""
    TC = TT // NC            # tokens per chunk == tokens per x-shard
    MT = TC // P             # token tiles per chunk
    KD = DM // P             # d_model k-blocks
    FFL = FF // NC           # local ff shard
    FFK = FFL // P           # ff k-blocks (phase 3 contraction)
    NFH = max(1, FFL // W)   # ff halves per G/U psum group
    WF = min(W, FFL)
    ND3 = max(1, DM // W)    # dm quarters of the P3 output
    W3 = min(W, DM)
    NPC = min(4, MT)         # xqT AllGather pieces per chunk
    TPQ = TC // NPC
    MTQ = TPQ // P           # token tiles per piece
    NW = float(FF * DM)      # elements per full weight matrix
    TC8 = TC // NC           # RS output rows per chunk per core
    rg = [list(range(NC))]
    assert FFK <= 8 and TPQ % P == 0

    nc = bacc.Bacc(
        "TRN2",
        target_bir_lowering=False,
        debug=False,
        enable_asserts=False,
        num_devices=NC,
    )

    x_d = nc.dram_tensor("x", [TC, DM], F32, kind="ExternalInput")
    wg_d = nc.dram_tensor("wg", [FFL, DM], F32, kind="ExternalInput")
    wu_d = nc.dram_tensor("wu", [FFL, DM], F32, kind="ExternalInput")
    wd_d = nc.dram_tensor("wd", [DM, FFL], F32, kind="ExternalInput")
    out_d = nc.dram_tensor("out_t", [NC * TC8, DM], F32, kind="ExternalOutput")

    with tile.TileContext(nc, num_cores=NC) as tc:
        with contextlib.ExitStack() as outer:
            dram = outer.enter_context(tc.tile_pool(name="dram", bufs=1, space="DRAM"))
            tiny = outer.enter_context(tc.tile_pool(name="tiny", bufs=1))

            # DRAM scratch (xqT is piece-major so every DMA packet is big)
            xqt_sh = [
                dram.tile([P, KD, TPQ], BF16, name=f"xqt_sh{q}") for q in range(NPC)
            ]
            xqt_all = [
                dram.tile([NC, P, KD, TPQ], BF16, name=f"xqt_all{q}",
                          addr_space="Shared")
                for q in range(NPC)
            ]
            sx_sh_d = dram.tile([P, MT], F32)
            sx_all_d = dram.tile([NC, P, MT], F32, addr_space="Shared")
            ws_in = dram.tile([1, 4], F32)
            ws_out = dram.tile([1, 4], F32, addr_space="Shared")
            hmax_in = dram.tile([NC, P, MT], F32)
            hmax_out = [
                dram.tile([P, MT], F32, name=f"hmax_out{i}", addr_space="Shared")
                for i in range(NC)
            ]
            pout_d = dram.tile([NC, TC, DM], BF16)
            rsout_d = dram.tile([NC, TC8, DM], BF16)

            # persistent small tiles
            ones_row = tiny.tile([1, P], F32)
            nc.vector.memset(ones_row, 1.0)
            ident = tiny.tile([P, P], BF16)
            make_identity(nc, ident)
            sb = tiny.tile([P, 8], F32)   # bcast: bgA,buA,bdA,-,swg,swu,swd,-
            NTT = NC * MT                 # total token tiles
            sx_sb = tiny.tile([P, NTT], F32)
            sxg_sb = tiny.tile([P, NTT], F32)
            sxu_sb = tiny.tile([P, NTT], F32)
            rh_sb = tiny.tile([P, NTT], F32)
            shd_sb = tiny.tile([P, NTT], F32)
            hmax_sb = tiny.tile([P, NTT], F32)

            # persistent ternary weights (bf16, transposed for matmul)
            wgt_sb = tiny.tile([P, KD, FFL], BF16)
            wut_sb = tiny.tile([P, KD, FFL], BF16)
            wdt_sb = tiny.tile([P, FFK, DM], BF16)

            def pe_transpose(src, nblk, dst3, ps_pool):
                """src [P, nblk*P] bf16 -> dst3 [P, nblk, P] (3D slice),
                via PE-transpose through PSUM in groups of <=8 blocks."""
                for h0 in range(0, nblk, 8):
                    nb = min(8, nblk - h0)
                    ps = ps_pool.tile([P, 8, P], BF16, name="ps_tr")
                    for j in range(nb):
                        nc.tensor.transpose(
                            ps[:, j, :], src[:, (h0 + j) * P : (h0 + j + 1) * P],
                            ident,
                        )
                    nc.vector.tensor_copy(
                        dst3[:, h0 : h0 + nb, :], ps[:, :nb, :]
                    )

            # ------------- prologue: x-quant shard + AG, weight scales,
            # ------------- ternarize weights into SBUF
            with contextlib.ExitStack() as pro:
                pspro = pro.enter_context(
                    tc.tile_pool(name="pspro", bufs=3, space="PSUM")
                )
                ps0 = pro.enter_context(
                    tc.tile_pool(name="ps0", bufs=1, space="PSUM")
                )
                xw_p = pro.enter_context(tc.tile_pool(name="xw", bufs=2))
                xtr_p = pro.enter_context(tc.tile_pool(name="xtr", bufs=2))
                s0_p = pro.enter_context(tc.tile_pool(name="s0", bufs=3))
                s0t_p = pro.enter_context(tc.tile_pool(name="s0t", bufs=4))
                wr_p = pro.enter_context(tc.tile_pool(name="wr", bufs=3))
                wt_p = pro.enter_context(tc.tile_pool(name="wt", bufs=2))

                # S0: weight scale sums (|w| over shard, AllReduce later)
                acc3 = tiny.tile([P, 4], F32)
                nc.vector.memset(acc3, 0.0)
                for src, col, rows, cols in (
                    (wg_d, 0, FFL, DM),
                    (wu_d, 1, FFL, DM),
                    (wd_d, 2, DM, FFL),
                ):
                    for r0 in range(0, rows, P):
                        t_in = s0_p.tile([P, DM], F32, name="s0raw")
                        nc.sync.dma_start(t_in[:, :cols], src[r0 : r0 + P, :])
                        t_sum = s0t_p.tile([P, 1], F32, name="s0sum")
                        nc.scalar.activation(
                            out=t_in[:, :cols], in_=t_in[:, :cols],
                            func=AFT.Abs, accum_out=t_sum,
                        )
                        nc.vector.tensor_tensor(
                            out=acc3[:, col : col + 1],
                            in0=acc3[:, col : col + 1],
                            in1=t_sum,
                            op=ADD,
                        )
                ones_col = s0t_p.tile([P, 1], F32, name="ones_col")
                nc.vector.memset(ones_col, 1.0)
                ps_s = ps0.tile([P, W], F32, name="ps_s")
                nc.tensor.matmul(
                    ps_s[:4, :1], acc3[:, :4], ones_col, start=True, stop=True
                )
                sb_s = s0t_p.tile([4, 1], F32, name="sb_s")
                nc.vector.tensor_copy(sb_s, ps_s[:4, :1])
                nc.gpsimd.dma_start(ws_in[0, :4], sb_s[:, 0])
                nc.gpsimd.collective_compute(
                    "AllReduce",
                    ADD,
                    replica_groups=rg,
                    ins=[ws_in[:].opt()],
                    outs=[ws_out[:].opt()],
                )
                sums_row = s0t_p.tile([1, 4], F32, name="sums_row")
                nc.gpsimd.dma_start(sums_row, ws_out[:])
                sw_row = s0t_p.tile([1, 4], F32, name="sw_row")
                nc.vector.tensor_scalar(
                    out=sw_row, in0=sums_row, scalar1=1.0 / NW, scalar2=EPS,
                    op0=MULT, op1=ADD,
                )
                beta_row = s0t_p.tile([1, 4], F32, name="beta_row")
                nc.vector.reciprocal(beta_row, sw_row)
                row8 = s0t_p.tile([1, 8], F32, name="row8")
                nc.vector.tensor_scalar(
                    out=row8[:, 0:4], in0=beta_row, scalar1=ALPHA, scalar2=None,
                    op0=MULT, op1=BYP,
                )
                nc.vector.tensor_copy(row8[:, 4:8], sw_row)
                ps_b = ps0.tile([P, W], F32, name="ps_b")
                nc.tensor.matmul(ps_b[:, :8], ones_row, row8, start=True, stop=True)
                nc.vector.tensor_copy(sb, ps_b[:, :8])

                # X-quant of own token shard -> transposed bf16 pieces -> AG
                sxl = tiny.tile([P, MT], F32)
                xtr_tiles = {}
                for m in range(MT):
                    q, mrel = m // MTQ, m % MTQ
                    if mrel == 0:
                        xtr_tiles[q] = xtr_p.tile([P, KD, TPQ], BF16, name="xtr")
                    xt = xw_p.tile([P, DM], F32, name="xt")
                    nc.sync.dma_start(xt, x_d[m * P : (m + 1) * P, :])
                    amax = s0t_p.tile([P, 1], F32, name="amax")
                    nc.vector.tensor_reduce(
                        amax, xt, axis=AXX, op=MAX, apply_absolute_value=True
                    )
                    nc.vector.tensor_scalar(
                        out=sxl[:, m : m + 1], in0=amax, scalar1=EPS,
                        scalar2=1.0 / 127.0, op0=MAX, op1=MULT,
                    )
                    rxc = s0t_p.tile([P, 1], F32, name="rxc")
                    nc.vector.reciprocal(rxc, sxl[:, m : m + 1])
                    nc.vector.tensor_scalar(
                        out=xt, in0=xt, scalar1=rxc, scalar2=CR, op0=MULT, op1=ADD,
                    )
                    xq = xw_p.tile([P, DM], BF16, name="xq")
                    nc.vector.tensor_scalar(
                        out=xq, in0=xt, scalar1=CR, scalar2=None, op0=SUB, op1=BYP,
                    )
                    for h0 in range(0, KD, 8):
                        nb = min(8, KD - h0)
                        ps = pspro.tile([P, 8, P], BF16, name="ps_tr")
                        for j in range(nb):
                            nc.tensor.transpose(
                                ps[:, j, :], xq[:, (h0 + j) * P : (h0 + j + 1) * P],
                                ident,
                            )
                        nc.vector.tensor_copy(
                            xtr_tiles[q][:, h0 : h0 + nb, mrel * P : (mrel + 1) * P],
                            ps[:, :nb, :],
                        )
                    if mrel == MTQ - 1:
                        nc.sync.dma_start(xqt_sh[q][:], xtr_tiles[q])
                nc.gpsimd.dma_start(sx_sh_d[:], sxl)
                nc.gpsimd.collective_compute(
                    "AllGather",
                    BYP,
                    replica_groups=rg,
                    ins=[sx_sh_d[:].opt()],
                    outs=[sx_all_d[:].opt()],
                )
                # sx readback for all tokens -> per-partition columns
                # (before the big xqt AGs so sigmoid scales aren't gated
                # behind them on the serialized collective queue)
                for i in range(NC):
                    nc.sync.dma_start(
                        sx_sb[:, i * MT : (i + 1) * MT], sx_all_d[i]
                    )
                nc.vector.tensor_scalar(
                    out=sxg_sb, in0=sx_sb, scalar1=sb[:, 4:5], scalar2=None,
                    op0=MULT, op1=BYP,
                )
                nc.vector.tensor_scalar(
                    out=sxu_sb, in0=sx_sb, scalar1=sb[:, 5:6], scalar2=None,
                    op0=MULT, op1=BYP,
                )
                for q in range(NPC):
                    nc.gpsimd.collective_compute(
                        "AllGather",
                        BYP,
                        replica_groups=rg,
                        ins=[xqt_sh[q][:].opt()],
                        outs=[xqt_all[q][:].opt()],
                    )

                # ternarize weights into SBUF (transposed bf16, PE transpose)
                def ternarize(src, beta_col, dst, nblk, rows):
                    cols = nblk * P
                    for r0 in range(0, rows, P):
                        raw = wr_p.tile([P, DM], F32, name="wraw")
                        nc.sync.dma_start(raw[:, :cols], src[r0 : r0 + P, :])
                        nc.scalar.activation(
                            out=raw[:, :cols], in_=raw[:, :cols], func=AFT.Tanh,
                            scale=sb[:, beta_col : beta_col + 1],
                        )
                        tern = wt_p.tile([P, DM], BF16, name="wtern")
                        nc.vector.tensor_scalar(
                            out=tern[:, :cols], in0=raw[:, :cols], scalar1=CR,
                            scalar2=CR, op0=ADD, op1=SUB,
                        )
                        pe_transpose(tern, nblk, dst[:, :, r0 : r0 + P], pspro)

                ternarize(wg_d, 0, wgt_sb, KD, FFL)
                ternarize(wu_d, 1, wut_sb, KD, FFL)
                ternarize(wd_d, 2, wdt_sb, FFK, DM)

            # ------------- main pipeline over 8 token chunks -------------
            with contextlib.ExitStack() as mn:
                psum = mn.enter_context(
                    tc.tile_pool(name="psum", bufs=7, space="PSUM")
                )
                pstr = mn.enter_context(
                    tc.tile_pool(name="pstr", bufs=1, space="PSUM")
                )
                xqc_p = mn.enter_context(tc.tile_pool(name="xqc", bufs=2))
                hp_p = mn.enter_context(tc.tile_pool(name="hp", bufs=MT + 5))
                gt_p = mn.enter_context(tc.tile_pool(name="gt", bufs=2))
                hq_p = mn.enter_context(tc.tile_pool(name="hq", bufs=2))
                hqt_p = mn.enter_context(tc.tile_pool(name="hqt", bufs=4))
                st_p = mn.enter_context(tc.tile_pool(name="st", bufs=2))
                sc_p = mn.enter_context(tc.tile_pool(name="scp", bufs=4))

                hp_tiles = {}
                xqc_tiles = {}

                def load_xqc(i, q):
                    t = xqc_p.tile([P, KD, TPQ], BF16, name="xqc")
                    nc.sync.dma_start(t, xqt_all[q][i])
                    xqc_tiles[(i, q)] = t

                def phase1_tile(i, m):
                    g = i * MT + m
                    xq_t = xqc_tiles[(i, m // MTQ)]
                    trel = (m % MTQ) * P
                    psG = [psum.tile([P, W], F32, name="ps_main") for _ in range(NFH)]
                    psU = [psum.tile([P, W], F32, name="ps_main") for _ in range(NFH)]
                    for k in range(KD):
                        lhsT = xq_t[:, k, trel : trel + P]
                        st, sp = (k == 0), (k == KD - 1)
                        for f in range(NFH):
                            nc.tensor.matmul(
                                psG[f][:, :WF], lhsT,
                                wgt_sb[:, k, f * WF : (f + 1) * WF],
                                start=st, stop=sp,
                            )
                        for f in range(NFH):
                            nc.tensor.matmul(
                                psU[f][:, :WF], lhsT,
                                wut_sb[:, k, f * WF : (f + 1) * WF],
                                start=st, stop=sp,
                            )
                    hp = hp_p.tile([P, FFL], F32, name="hp")
                    for f in range(NFH):
                        gt = gt_p.tile([P, WF], F32, name="gt")
                        nc.scalar.activation(
                            out=gt, in_=psG[f][:, :WF], func=AFT.Sigmoid,
                            scale=sxg_sb[:, g : g + 1],
                        )
                        nc.vector.tensor_tensor(
                            out=hp[:, f * WF : (f + 1) * WF], in0=gt,
                            in1=psU[f][:, :WF], op=MULT,
                        )
                    nc.vector.tensor_reduce(
                        hmax_sb[:, g : g + 1], hp, axis=AXX, op=MAX,
                        apply_absolute_value=True,
                    )
                    hp_tiles[(i, m)] = hp

                def chunk_absmax_ar(i):
                    nc.gpsimd.dma_start(
                        hmax_in[i], hmax_sb[:, i * MT : (i + 1) * MT]
                    )
                    nc.gpsimd.collective_compute(
                        "AllReduce",
                        MAX,
                        replica_groups=rg,
                        ins=[hmax_in[i].opt()],
                        outs=[hmax_out[i][:].opt()],
                    )

                amg_tiles = {}

                def chunk_amg(i):
                    # emitted right after maxAR(i) and BEFORE the next RS on
                    # the gpsimd queue: never blocked by a ReduceScatter, and
                    # keeps the AR wait off the sync queue (xqc prefetch)
                    amg = sc_p.tile([P, MT], F32, name="amg")
                    nc.gpsimd.dma_start(amg, hmax_out[i][:])
                    amg_tiles[i] = amg

                def chunk_scales(i):
                    amg = amg_tiles.pop(i)
                    cs = slice(i * MT, (i + 1) * MT)
                    ah = sc_p.tile([P, MT], F32, name="ah")
                    nc.vector.tensor_tensor(
                        out=ah, in0=amg, in1=sxu_sb[:, cs], op=MULT
                    )
                    sh = sc_p.tile([P, MT], F32, name="sh")
                    nc.vector.tensor_scalar(
                        out=sh, in0=ah, scalar1=EPS, scalar2=1.0 / 127.0,
                        op0=MAX, op1=MULT,
                    )
                    rs_t = sc_p.tile([P, MT], F32, name="rs_t")
                    nc.vector.reciprocal(rs_t, sh)
                    nc.vector.tensor_tensor(
                        out=rh_sb[:, cs], in0=rs_t, in1=sxu_sb[:, cs], op=MULT
                    )
                    nc.vector.tensor_scalar(
                        out=shd_sb[:, cs], in0=sh, scalar1=sb[:, 6:7], scalar2=None,
                        op0=MULT, op1=BYP,
                    )

                def quant_tile(i, m):
                    g = i * MT + m
                    hp = hp_tiles.pop((i, m))
                    nc.scalar.activation(
                        out=hp, in_=hp, func=AFT.Copy,
                        scale=rh_sb[:, g : g + 1], bias=CR,
                    )
                    hq = hq_p.tile([P, FFL], BF16, name="hq")
                    nc.scalar.activation(out=hq, in_=hp, func=AFT.Copy, bias=-CR)
                    ps = pstr.tile([P, FFK, P], BF16, name="ps_hqt")
                    for j in range(FFK):
                        nc.tensor.transpose(
                            ps[:, j, :], hq[:, j * P : (j + 1) * P], ident
                        )
                    hqt = hqt_p.tile([P, FFK, P], BF16, name="hqt")
                    nc.scalar.activation(out=hqt, in_=ps, func=AFT.Copy)
                    return hqt

                def phase3_tile(i, m, hqt):
                    g = i * MT + m
                    stg = st_p.tile([P, DM], BF16, name="stg")
                    for d0 in range(0, ND3, 2):
                        nd = min(2, ND3 - d0)
                        ps3 = [
                            psum.tile([P, W], F32, name="ps_main")
                            for _ in range(nd)
                        ]
                        for b in range(FFK):
                            lhsT = hqt[:, b, :]
                            st, sp = (b == 0), (b == FFK - 1)
                            for d in range(nd):
                                nc.tensor.matmul(
                                    ps3[d][:, :W3], lhsT,
                                    wdt_sb[:, b, (d0 + d) * W3 : (d0 + d + 1) * W3],
                                    start=st, stop=sp,
                                )
                        for d in range(nd):
                            nc.vector.tensor_scalar(
                                out=stg[:, (d0 + d) * W3 : (d0 + d + 1) * W3],
                                in0=ps3[d][:, :W3],
                                scalar1=shd_sb[:, g : g + 1], scalar2=None,
                                op0=MULT, op1=BYP,
                            )
                    nc.scalar.dma_start(
                        pout_d[i, m * P : (m + 1) * P, :], stg
                    )

                pending_copies = []
                rsin_p = mn.enter_context(tc.tile_pool(name="rsin", bufs=1))
                cvt_p = mn.enter_context(tc.tile_pool(name="cvt", bufs=1))

                def emit_out_copy(j):
                    rsin = rsin_p.tile([TC8, DM], BF16, name="rsin")
                    nc.scalar.dma_start(rsin, rsout_d[j][:])
                    cvt = cvt_p.tile([TC8, DM], F32, name="cvt")
                    nc.scalar.activation(out=cvt, in_=rsin, func=AFT.Copy)
                    nc.scalar.dma_start(out_d[j * TC8 : (j + 1) * TC8, :], cvt)

                def chunk_rs(i):
                    # out-copies of finished chunks ride the scalar queue and
                    # are emitted a chunk later so they never wait on the RS
                    while pending_copies:
                        emit_out_copy(pending_copies.pop())
                    nc.gpsimd.collective_compute(
                        "ReduceScatter",
                        ADD,
                        replica_groups=rg,
                        ins=[pout_d[i].opt()],
                        outs=[rsout_d[i].opt()],
                    )
                    pending_copies.append(i)

                # Software pipeline over global tile index gg.  Chunk j's
                # quant+P3 tiles are burst-scheduled into the last MT-3
                # iterations of chunk j+1 (2,2,2,1,...), so:
                #  - the absmax-AR(j) result has 3 tiles of slack before
                #    anything consumes it (no FIFO blocks on AR latency),
                #  - P3(j) finishes exactly at chunk j+2's start, where
                #    RS(j) is emitted — it then has a full chunk period on
                #    the gpsimd queue before maxAR(j+2) needs it.
                NTT_ = NC * MT
                # per-iteration assignment: work[off] = list of tile indices m
                assert MT >= 6 or MT == 1
                if MT >= 6:
                    counts = [2] * 3 + [1] * (MT - 6)
                    offs = list(range(3, MT))
                else:  # tiny correctness-only shapes: flat lag schedule
                    counts = [1]
                    offs = [3]
                sched = {}  # iteration gg -> list of (chunk, m)
                for j in range(NC):
                    base = (j + 1) * MT
                    mm = 0
                    for off, cnt in zip(offs, counts):
                        for _ in range(cnt):
                            if mm < MT:
                                sched.setdefault(base + off, []).append((j, mm))
                                mm += 1
                    while mm < MT:  # MT==1 fallback spill
                        sched.setdefault(base + offs[-1] + mm, []).append((j, mm))
                        mm += 1
                p3_last = {}
                for gg2, lst in sched.items():
                    for (j, _m) in lst:
                        p3_last[j] = max(p3_last.get(j, 0), gg2)
                rs_at = {}
                for j in range(NC):
                    rs_at.setdefault(
                        max((j + 2) * MT, p3_last[j] + 1), []
                    ).append(j)
                load_xqc(0, 0)
                last_gg = max(max(sched), max(rs_at)) + 1
                for gg in range(max(NTT_, last_gg) + 1):
                    if gg % MT == 0 and 1 <= gg // MT <= NC:
                        chunk_amg(gg // MT - 1)
                    for j in rs_at.get(gg, ()):
                        chunk_rs(j)
                    if gg < NTT_:
                        i, m = gg // MT, gg % MT
                        if m % MTQ == 0:
                            nq, ni = m // MTQ + 1, i
                            if nq == NPC:
                                nq, ni = 0, i + 1
                            if ni <= NC - 1:
                                load_xqc(ni, nq)
                        phase1_tile(i, m)
                        if m == MT - 1:
                            chunk_absmax_ar(i)
                    gs = gg - (MT + 2)
                    if gs >= 0 and gs % MT == 0 and gs // MT < NC:
                        chunk_scales(gs // MT)
                    for (j, m) in sched.get(gg, ()):
                        hqt = quant_tile(j, m)
                        phase3_tile(j, m, hqt)
                for j in pending_copies:
                    emit_out_copy(j)

    nc.compile()
    return nc


_CACHE = {}
TRACE = False
LAST_RESULTS = None


def _get_program(TT, DM, FF, NC):
    key = (TT, DM, FF, NC)
    if key not in _CACHE:
        _CACHE[key] = build_program(TT, DM, FF, NC)
    return _CACHE[key]


def kernel(x, w_gate, w_up, w_down):
    from concourse.bass_utils import run_bass_kernel_spmd

    x = np.asarray(x, dtype=np.float32)
    w_gate = np.ascontiguousarray(np.asarray(w_gate, dtype=np.float32))
    w_up = np.ascontiguousarray(np.asarray(w_up, dtype=np.float32))
    w_down = np.ascontiguousarray(np.asarray(w_down, dtype=np.float32))

    B, S, DM = x.shape
    FF = w_gate.shape[0]
    NC = 8
    TT = B * S
    TC = TT // NC
    FFL = FF // NC
    TC8 = TC // NC

    xf = np.ascontiguousarray(x.reshape(TT, DM))
    nc = _get_program(TT, DM, FF, NC)

    in_maps = []
    for c in range(NC):
        in_maps.append(
            {
                "x": np.ascontiguousarray(xf[c * TC : (c + 1) * TC]),
                "wg": np.ascontiguousarray(w_gate[c * FFL : (c + 1) * FFL]),
                "wu": np.ascontiguousarray(w_up[c * FFL : (c + 1) * FFL]),
                "wd": np.ascontiguousarray(w_down[:, c * FFL : (c + 1) * FFL]),
            }
        )

    res = run_bass_kernel_spmd(
        nc, in_maps, core_ids=list(range(NC)), trace=TRACE
    )
    global LAST_RESULTS
    LAST_RESULTS = res
    # core c, chunk i holds tokens i*TC + c*TC8 + [0, TC8)
    out = np.empty((TT, DM), dtype=np.float32)
    for c in range(NC):
        rc = res.results[c]["out_t"].reshape(NC, TC8, DM)
        for i in range(NC):
            t0 = i * TC + c * TC8
            out[t0 : t0 + TC8] = rc[i]
    return out.reshape(B, S, DM)
